# revision 1
# baseline (speedup 1.0000x reference)
"""Trainium2 Bass kernel for nn_MinimizeEnergy (bond/angle/dihedral energies).

Strategy (per sharding hint): data-parallel over the term axis. Host sorts
terms by base atom index (HBM gather locality), shards equal counts across
8 cores, replicates pos. Each core indirect-DMA-gathers the contiguous
pos rows for its terms (indices are base+arange per reference construction),
computes per-term energies on DVE/ACT, accumulates per-partition partial
sums, and the host combines in float64.

Self-contained: only imports the installed concourse toolchain.
"""
import sys
for _p in ('/opt/trn_rl_repo',):
    if _p not in sys.path:
        sys.path.insert(0, _p)

import numpy as np
from contextlib import ExitStack

import concourse.bass as bass
import concourse.tile as tile
from concourse import bacc, mybir
from concourse.bass import IndirectOffsetOnAxis

F32 = mybir.dt.float32
I32 = mybir.dt.int32
AF = mybir.ActivationFunctionType
ALU = mybir.AluOpType
AX = mybir.AxisListType
PI = float(np.pi)
P = 128
N_CORES = 8

N_ATOMS = 2_000_000
N_BONDS = 2_000_000
N_ANGLES = 4_000_000
N_DIH = 2_000_000

TF = 768          # terms per partition per tile
CLIP = 1.0 - 1e-7


def _tile_plan(n_per_core):
    """List of per-tile TF values covering ceil(n/128) columns."""
    cols = -(-n_per_core // P)
    plan = []
    while cols > 0:
        t = min(TF, cols)
        plan.append(t)
        cols -= t
    return plan


def build_kernel(nb, na, nd):
    """nb/na/nd: per-core padded term counts (multiples of 128)."""
    nc = bacc.Bacc("TRN2", target_bir_lowering=False, debug=False,
                   num_devices=N_CORES)
    b_xyz = nc.dram_tensor("b_xyz", [P, (nb // P) * 6], F32, kind="ExternalInput").ap()
    b_eq = nc.dram_tensor("b_eq", [P, nb // P], F32, kind="ExternalInput").ap()
    b_tol = nc.dram_tensor("b_tol", [P, nb // P], F32, kind="ExternalInput").ap()
    a_xyz = nc.dram_tensor("a_xyz", [P, (na // P) * 9], F32, kind="ExternalInput").ap()
    a_eq = nc.dram_tensor("a_eq", [P, na // P], F32, kind="ExternalInput").ap()
    a_tol = nc.dram_tensor("a_tol", [P, na // P], F32, kind="ExternalInput").ap()
    d_xyz = nc.dram_tensor("d_xyz", [P, (nd // P) * 12], F32, kind="ExternalInput").ap()
    d_eq = nc.dram_tensor("d_eq", [P, nd // P], F32, kind="ExternalInput").ap()
    partials = nc.dram_tensor("partials", [P, 4], F32, kind="ExternalOutput").ap()

    with tile.TileContext(nc) as tc, ExitStack() as ctx:
        io = ctx.enter_context(tc.tile_pool(name="io", bufs=6))
        gp = ctx.enter_context(tc.tile_pool(name="gp", bufs=2))
        pl = ctx.enter_context(tc.tile_pool(name="pl", bufs=6))
        sm = ctx.enter_context(tc.tile_pool(name="sm", bufs=14))
        accp = ctx.enter_context(tc.tile_pool(name="accp", bufs=1))

        acc = accp.tile([P, 4], F32)
        nc.vector.memset(acc[:], 0.0)
        halfpi = accp.tile([P, 1], F32)
        nc.vector.memset(halfpi[:], PI / 2)
        epsb = accp.tile([P, 1], F32)
        nc.vector.memset(epsb[:], 1e-6)

        def vec(shape_tf, n=3, tag=None):
            return pl.tile([P, shape_tf, n], F32, tag="v3", name=tag or "v3")

        def plane(shape_tf, tag=None):
            return sm.tile([P, shape_tf], F32, tag="pln", name=tag or "pln")

        def load(dram_ap, col0, tf, dtype):
            t = io.tile([P, tf], dtype, tag="io", name="iot")
            nc.gpsimd.dma_start(t[:], dram_ap[:, col0:col0 + tf])
            return t

        def gather(xyz_ap, col0, tf, elem):
            G = gp.tile([P, tf, elem], F32, tag="G", name="G")
            nc.gpsimd.dma_start(G[:].bitcast(F32), xyz_ap[:, col0 * elem:(col0 + tf) * elem])
            return G

        def accum(col, e_plane, tf):
            # acc[:, col] += sum over free axis of e_plane
            s = sm.tile([P, 1], F32, tag="acc_s", name="acc_s")
            nc.vector.tensor_reduce(s[:], e_plane[:], axis=AX.X, op=ALU.add)
            nc.vector.tensor_add(acc[:, col:col + 1], acc[:, col:col + 1], s[:])

        # ---------------- bonds ----------------
        col = 0
        for tf in _tile_plan(nb):
            te = load(b_eq, col, tf, F32)
            tt = load(b_tol, col, tf, F32)
            G = gather(b_xyz, col, tf, 6)
            D = vec(tf, 3, tag="bD")
            nc.vector.tensor_sub(D[:], G[:, :, 0:3], G[:, :, 3:6])
            S = vec(tf, 3, tag="bS")
            nc.scalar.activation(S[:], D[:], AF.Square)
            n2 = plane(tf, tag="bn2")
            nc.vector.tensor_reduce(n2[:], S[:], axis=AX.X, op=ALU.add)
            d = plane(tf, tag="bd")
            nc.scalar.activation(d[:], n2[:], AF.Sqrt)
            diff = plane(tf, tag="bdiff")
            nc.vector.tensor_sub(diff[:], d[:], te[:])
            df2 = plane(tf, tag="bdf2")
            nc.scalar.activation(df2[:], diff[:], AF.Square)
            tl2 = plane(tf, tag="btl2")
            nc.scalar.activation(tl2[:], tt[:], AF.Square)
            t0 = plane(tf, tag="bt0")
            nc.vector.tensor_sub(t0[:], df2[:], tl2[:])
            e = plane(tf, tag="be")
            nc.vector.tensor_scalar(e[:], t0[:], 0.0, None, ALU.max)
            accum(0, e, tf)
            col += tf

        # ---------------- angles ----------------
        col = 0
        for tf in _tile_plan(na):
            te = load(a_eq, col, tf, F32)
            tt = load(a_tol, col, tf, F32)
            G = gather(a_xyz, col, tf, 9)
            B0 = vec(tf, 3, tag="aB0")
            nc.vector.tensor_sub(B0[:], G[:, :, 0:3], G[:, :, 3:6])
            B1 = vec(tf, 3, tag="aB1")
            nc.gpsimd.tensor_sub(B1[:], G[:, :, 6:9], G[:, :, 3:6])
            PM = vec(tf, 3, tag="aPM")
            nc.gpsimd.tensor_mul(PM[:], B0[:], B1[:])
            d01 = plane(tf, tag="ad01")
            nc.vector.tensor_reduce(d01[:], PM[:], axis=AX.X, op=ALU.add)
            S0 = vec(tf, 3, tag="aS0")
            nc.scalar.activation(S0[:], B0[:], AF.Square)
            n0 = plane(tf, tag="an0")
            nc.vector.tensor_reduce(n0[:], S0[:], axis=AX.X, op=ALU.add)
            S1 = vec(tf, 3, tag="aS1")
            nc.scalar.activation(S1[:], B1[:], AF.Square)
            n1 = plane(tf, tag="an1")
            nc.vector.tensor_reduce(n1[:], S1[:], axis=AX.X, op=ALU.add)
            nn = plane(tf, tag="ann")
            nc.vector.tensor_mul(nn[:], n0[:], n1[:])
            s = plane(tf, tag="as")
            nc.scalar.activation(s[:], nn[:], AF.Sqrt)
            rs = plane(tf, tag="ars")
            nc.vector.reciprocal_approx_fast(rs[:], s[:])
            c = plane(tf, tag="ac")
            nc.vector.tensor_mul(c[:], d01[:], rs[:])
            nc.vector.tensor_scalar(c[:], c[:], -CLIP, CLIP, ALU.max, ALU.min)
            ac_ = plane(tf, tag="aabs")  # |c|
            nc.vector.scalar_tensor_tensor(ac_[:], c[:], -1.0, c[:], ALU.mult, ALU.max)
            mn = plane(tf, tag="amn")    # 1 - |c|
            nc.vector.tensor_scalar(mn[:], ac_[:], -1.0, 1.0, ALU.mult, ALU.add)
            mx = plane(tf, tag="amx")    # 1 + |c|
            nc.vector.tensor_scalar(mx[:], ac_[:], 1.0, None, ALU.add)
            rmx = plane(tf, tag="armx")
            nc.vector.reciprocal_approx_fast(rmx[:], mx[:])
            r = plane(tf, tag="ar")
            nc.vector.tensor_mul(r[:], mn[:], rmx[:])
            m = plane(tf, tag="am")
            nc.scalar.activation(m[:], r[:], AF.Sqrt)
            a = plane(tf, tag="aa")
            nc.scalar.activation(a[:], m[:], AF.Arctan)
            # theta = 2a  (c>=0)  |  pi - 2a  (c<0)  -> 2a + mask*(pi - 4a)
            msk = plane(tf, tag="amsk")
            nc.gpsimd.tensor_scalar(msk[:], c[:], 0.0, None, ALU.is_lt)
            pa = plane(tf, tag="apa")
            nc.gpsimd.tensor_scalar(pa[:], a[:], -4.0, PI, ALU.mult, ALU.add)
            pm2 = plane(tf, tag="apm2")
            nc.gpsimd.tensor_mul(pm2[:], msk[:], pa[:])
            th = plane(tf, tag="ath")
            nc.vector.scalar_tensor_tensor(th[:], a[:], 2.0, pm2[:], ALU.mult, ALU.add)
            diff = plane(tf, tag="adiff")
            nc.vector.tensor_sub(diff[:], th[:], te[:])
            df2 = plane(tf, tag="adf2")
            nc.scalar.activation(df2[:], diff[:], AF.Square)
            tl2 = plane(tf, tag="atl2")
            nc.scalar.activation(tl2[:], tt[:], AF.Square)
            t0 = plane(tf, tag="at0")
            nc.vector.tensor_sub(t0[:], df2[:], tl2[:])
            e = plane(tf, tag="ae")
            nc.vector.tensor_scalar(e[:], t0[:], 0.0, None, ALU.max)
            accum(1, e, tf)
            col += tf

        # ---------------- dihedrals ----------------
        # cos(dih) = X/sqrt(X^2+L^2 Y^2), sin(dih) = L*Y/sqrt(X^2+L^2 Y^2)
        # X = L^2 (b0.b2) - (b0.u)(b2.u), Y = (u x b0).b2, u = p2-p1, L^2=u.u
        # energy = 2 - 2*cos(dih - eq); accumulate cos(dih-eq) only.
        col = 0
        for tf in _tile_plan(nd):
            te = load(d_eq, col, tf, F32)
            G = gather(d_xyz, col, tf, 12)
            B0 = vec(tf, 3, tag="dB0")
            nc.vector.tensor_sub(B0[:], G[:, :, 0:3], G[:, :, 3:6])
            U = vec(tf, 3, tag="dU")
            nc.vector.tensor_sub(U[:], G[:, :, 6:9], G[:, :, 3:6])
            B2 = vec(tf, 3, tag="dB2")
            nc.gpsimd.tensor_sub(B2[:], G[:, :, 9:12], G[:, :, 6:9])
            PM = vec(tf, 3, tag="dPM")
            nc.vector.tensor_mul(PM[:], B0[:], B2[:])
            b0b2 = plane(tf, tag="db0b2")
            nc.vector.tensor_reduce(b0b2[:], PM[:], axis=AX.X, op=ALU.add)
            nc.vector.tensor_mul(PM[:], B0[:], U[:])
            b0u = plane(tf, tag="db0u")
            nc.vector.tensor_reduce(b0u[:], PM[:], axis=AX.X, op=ALU.add)
            PMb = vec(tf, 3, tag="dPMb")
            nc.gpsimd.tensor_mul(PMb[:], B2[:], U[:])
            b2u = plane(tf, tag="db2u")
            nc.vector.tensor_reduce(b2u[:], PMb[:], axis=AX.X, op=ALU.add)
            SU = vec(tf, 3, tag="dSU")
            nc.scalar.activation(SU[:], U[:], AF.Square)
            L2 = plane(tf, tag="dL2")
            nc.vector.tensor_reduce(L2[:], SU[:], axis=AX.X, op=ALU.add)
            t1 = plane(tf, tag="dt1")
            nc.vector.tensor_mul(t1[:], L2[:], b0b2[:])
            t2 = plane(tf, tag="dt2")
            nc.vector.tensor_mul(t2[:], b0u[:], b2u[:])
            X = plane(tf, tag="dX")
            nc.vector.tensor_sub(X[:], t1[:], t2[:])
            # cross C = U x B0 (reuse PM as C)
            C = PM
            w1 = plane(tf, tag="dw1")
            w2 = plane(tf, tag="dw2")
            for k in range(3):
                i1, i2 = (k + 1) % 3, (k + 2) % 3
                nc.vector.tensor_mul(w1[:], U[:, :, i1], B0[:, :, i2])
                nc.vector.tensor_mul(w2[:], U[:, :, i2], B0[:, :, i1])
                nc.vector.tensor_sub(C[:, :, k], w1[:], w2[:])
            CB = vec(tf, 3, tag="dCB")
            nc.vector.tensor_mul(CB[:], C[:], B2[:])
            Y = plane(tf, tag="dY")
            nc.vector.tensor_reduce(Y[:], CB[:], axis=AX.X, op=ALU.add)
            X2 = plane(tf, tag="dX2")
            nc.scalar.activation(X2[:], X[:], AF.Square)
            Y2 = plane(tf, tag="dY2")
            nc.scalar.activation(Y2[:], Y[:], AF.Square)
            LY2 = plane(tf, tag="dLY2")
            nc.gpsimd.tensor_mul(LY2[:], L2[:], Y2[:])
            den = plane(tf, tag="dden")
            nc.gpsimd.tensor_add(den[:], X2[:], LY2[:])
            tden = plane(tf, tag="dtden")
            nc.scalar.activation(tden[:], den[:], AF.Sqrt, bias=epsb[:])
            rt = plane(tf, tag="drt")
            nc.vector.reciprocal_approx_fast(rt[:], tden[:])
            L = plane(tf, tag="dL")
            nc.scalar.activation(L[:], L2[:], AF.Sqrt)
            LY = plane(tf, tag="dLY")
            nc.vector.tensor_mul(LY[:], L[:], Y[:])
            aeq = plane(tf, tag="daeq")
            nc.scalar.activation(aeq[:], te[:], AF.Abs)
            seq = plane(tf, tag="dseq")
            nc.scalar.activation(seq[:], te[:], AF.Sin)
            ceq = plane(tf, tag="dceq")
            nc.scalar.activation(ceq[:], aeq[:], AF.Sin, scale=-1.0, bias=halfpi[:])
            nx = plane(tf, tag="dnx")
            nc.gpsimd.tensor_mul(nx[:], X[:], ceq[:])
            ny = plane(tf, tag="dny")
            nc.gpsimd.tensor_mul(ny[:], LY[:], seq[:])
            num = plane(tf, tag="dnum")
            nc.vector.tensor_add(num[:], nx[:], ny[:])
            cdd = plane(tf, tag="dcdd")
            nc.vector.tensor_mul(cdd[:], num[:], rt[:])
            accum(2, cdd, tf)
            col += tf

        nc.gpsimd.dma_start(partials[:], acc[:])
    nc.compile()
    return nc


def _run_spmd(nc, in_maps):
    import os
    if os.environ.get("EK_SIM") == "1":
        from concourse.bass_interp import CoreSim
        results = []
        for m in in_maps:
            sim = CoreSim(nc)
            for k, v in m.items():
                sim.tensor(k)[:] = v
            sim.simulate()
            results.append({"partials": np.array(sim.tensor("partials"))})
        return results
    from concourse.bass_utils import run_bass_kernel_spmd
    res = run_bass_kernel_spmd(nc, in_maps, list(range(len(in_maps))))
    return res.results


_BUILD_CACHE = {}


def _get_kernel(nb, na, nd):
    key = (nb, na, nd)
    if key not in _BUILD_CACHE:
        _BUILD_CACHE[key] = build_kernel(nb, na, nd)
    return _BUILD_CACHE[key]


def _prep_type(pos, idcs, eq, tol, n_per_core_pad, arity):
    """Host-side neighbor materialization: shard terms to 8 cores, pad,
    gather pos rows per term -> [P, cols*3*arity] coordinate array."""
    base = np.asarray(idcs)[:, 0].astype(np.int64)
    eq = np.asarray(eq, dtype=np.float32)
    tol = None if tol is None else np.asarray(tol, dtype=np.float32)
    n = base.shape[0]
    per = n // N_CORES
    outs = []
    for c in range(N_CORES):
        bb = base[c * per:(c + 1) * per]
        ee = eq[c * per:(c + 1) * per]
        tt = None if tol is None else tol[c * per:(c + 1) * per]
        npad = n_per_core_pad - per
        if npad:
            bb = np.concatenate([bb, np.zeros(npad, np.int64)])
            ee = np.concatenate([ee, np.zeros(npad, np.float32)])
            if tt is not None:
                # huge tolerance -> relu(...)=0 for padding terms
                tt = np.concatenate([tt, np.full(npad, 1e3, np.float32)])
        coords = pos[bb[:, None] + np.arange(arity)]          # [npc, arity, 3]
        coords = coords.reshape(P, -1, arity * 3)             # [P, cols, arity*3]
        outs.append((coords.reshape(P, -1),
                     ee.reshape(P, -1, order='C'),
                     None if tt is None else tt.reshape(P, -1, order='C')))
    return outs, per


def _pad128(n):
    return -(-n // P) * P


def _dihedral_np(p, eq):
    p0, p1, p2, p3 = p[0], p[1], p[2], p[3]
    b0, b1, b2 = p0 - p1, p2 - p1, p3 - p2
    b1 = b1 / np.linalg.norm(b1)
    v = b0 - np.dot(b0, b1) * b1
    w = b2 - np.dot(b2, b1) * b1
    x = np.dot(v, w)
    y = np.dot(np.cross(b1, v), w)
    return np.arctan2(y, x) - eq


def kernel(pos, bond_idcs, bond_eq_val, bond_tolerance,
           angle_idcs, angle_eq_val, angle_tolerance,
           dih_idcs, dih_eq_val):
    pos = np.asarray(pos, dtype=np.float32)
    nb = _pad128(N_BONDS // N_CORES)
    na = _pad128(N_ANGLES // N_CORES)
    nd = _pad128(N_DIH // N_CORES)

    bonds, _ = _prep_type(pos, bond_idcs, bond_eq_val, bond_tolerance, nb, 2)
    angles, _ = _prep_type(pos, angle_idcs, angle_eq_val, angle_tolerance, na, 3)
    dihs, _ = _prep_type(pos, dih_idcs, dih_eq_val, None, nd, 4)

    nc = _get_kernel(nb, na, nd)

    in_maps = []
    for c in range(N_CORES):
        bi, be, bt = bonds[c]
        ai, ae, at = angles[c]
        di, de, _ = dihs[c]
        in_maps.append({
            "b_xyz": bi, "b_eq": be, "b_tol": bt,
            "a_xyz": ai, "a_eq": ae, "a_tol": at,
            "d_xyz": di, "d_eq": de,
        })

    results = _run_spmd(nc, in_maps)

    bond_sum = 0.0
    angle_sum = 0.0
    cos_sum = 0.0
    for c in range(N_CORES):
        p = results[c]["partials"].astype(np.float64)
        bond_sum += p[:, 0].sum()
        angle_sum += p[:, 1].sum()
        cos_sum += p[:, 2].sum()

    # padding corrections
    npad_d_total = (nd - N_DIH // N_CORES) * N_CORES
    if npad_d_total:
        # dummy dih terms: idx=0, eq=0
        cdd_pad = np.cos(_dihedral_np(np.asarray(pos[0:4], dtype=np.float64), 0.0))
        cos_sum -= npad_d_total * cdd_pad
    # bond/angle padding contribute exactly 0 via the huge-tolerance trick

    bond_energy = 1000.0 * bond_sum / N_BONDS
    angle_energy = 150.0 * angle_sum / N_ANGLES
    dih_energy = (2.0 * N_DIH - 2.0 * cos_sum) / N_DIH
    total = bond_energy + angle_energy + dih_energy
    return (np.float32(total), np.float32(bond_energy),
            np.float32(angle_energy), np.float32(dih_energy))


if __name__ == "__main__":
    # tiny self-check via CoreSim on a small fabricated problem is in test.py
    pass



# revision 18
# speedup vs baseline: 2.2763x; 2.2763x over previous
"""Trainium2 Bass kernel for nn_MinimizeEnergy (bond/angle/dihedral energies).

Strategy: data-parallel over the term axis (8 cores, equal shards). Host
marshals the gather: per term it emits edge-difference vectors (p_i - p_j)
as scaled fp16 planes (planar SoA layout, one [P, cols] plane per vector
component), plus fp16 eq / tol^2 planes. The device kernel does all the
math: norms, half-angle arctan for bond angles, dihedral cos via the
X/Y trig-free formulation, energy terms, and per-partition accumulation.

Numerics: fp16 throughout the elementwise pipeline (DVE 2x mode), fp32
reductions. Vectors are pre-scaled (bonds/angles 1/16, dihedrals 1/32) so
all intermediates stay in fp16 range; the scales cancel in the angle/dih
ratios and are undone inside the bond sqrt's free scale slot.

ACT table sets are grouped into three phases (sqrt -> reciprocal -> trig)
to pay only three ACT_TABLE_LOADs.
"""
import sys
for _p in ('/opt/trn_rl_repo',):
    if _p not in sys.path:
        sys.path.insert(0, _p)

import numpy as np
from contextlib import ExitStack

import concourse.bass as bass
import concourse.tile as tile
from concourse import bacc, mybir

F32 = mybir.dt.float32
F16 = mybir.dt.float16
AF = mybir.ActivationFunctionType
ALU = mybir.AluOpType
AX = mybir.AxisListType
PI = float(np.pi)
P = 128
N_CORES = 8

N_ATOMS = 2_000_000
N_BONDS = 2_000_000
N_ANGLES = 4_000_000
N_DIH = 2_000_000

SB = 1.0 / 16.0   # bond vector prescale
SA = 1.0 / 16.0   # angle vector prescale
SD = 1.0 / 32.0   # dihedral vector prescale

PAD_TOL2 = 1.0e3  # tol^2 for padding terms -> relu(...) == 0


def _cols(n_per_core):
    """Columns per partition, padded so every plane is 4B-aligned (cols
    multiple of 4)."""
    c = -(-n_per_core // P)
    return -(-c // 4) * 4


def build_kernel(nb, na, nd):
    """nb/na/nd: per-core column counts (terms per partition). nb == nd
    is assumed by the buffer-sharing plan below."""
    nc = bacc.Bacc("TRN2", target_bir_lowering=False, debug=False,
                   num_devices=N_CORES)
    b_v = nc.dram_tensor("b_v", [P, 3 * nb], F16, kind="ExternalInput").ap()
    b_eq = nc.dram_tensor("b_eq", [P, nb], F16, kind="ExternalInput").ap()
    b_t2 = nc.dram_tensor("b_t2", [P, nb], F16, kind="ExternalInput").ap()
    a_v = nc.dram_tensor("a_v", [P, 6 * na], F16, kind="ExternalInput").ap()
    a_eq = nc.dram_tensor("a_eq", [P, na], F16, kind="ExternalInput").ap()
    a_t2 = nc.dram_tensor("a_t2", [P, na], F16, kind="ExternalInput").ap()
    d_v = nc.dram_tensor("d_v", [P, 9 * nd], F16, kind="ExternalInput").ap()
    d_eq = nc.dram_tensor("d_eq", [P, nd], F16, kind="ExternalInput").ap()
    partials = nc.dram_tensor("partials", [P, 4], F32, kind="ExternalOutput").ap()

    V = nc.vector      # DVE
    A = nc.scalar      # ACT
    G = nc.gpsimd      # Pool

    with tile.TileContext(nc) as tc, ExitStack() as ctx:
        pers = ctx.enter_context(tc.tile_pool(name="pers", bufs=1))
        ainp = ctx.enter_context(tc.tile_pool(name="ainp", bufs=1))
        dinp = ctx.enter_context(tc.tile_pool(name="dinp", bufs=1))

        acc = pers.tile([P, 4], F32)
        V.memset(acc[:], 0.0)
        halfpi = pers.tile([P, 1], F32)
        V.memset(halfpi[:], PI / 2)

        # ---- input tiles + DMA (issued up front; Pool engine triggers) ----
        ta_v = ainp.tile([P, 6 * na], F16)
        ta_eq = ainp.tile([P, na], F16)
        ta_t2 = ainp.tile([P, na], F16)
        td_v = dinp.tile([P, 9 * nd], F16)
        td_eq = dinp.tile([P, nd], F16)

        # ================= BONDS (own pool scope, freed after) =============
        with tc.tile_pool(name="binp", bufs=1) as binp:
            tb_v = binp.tile([P, 3 * nb], F16)
            tb_eq = binp.tile([P, nb], F16)
            tb_t2 = binp.tile([P, nb], F16)
            G.dma_start(tb_v[:], b_v[:, :])
            G.dma_start(tb_eq[:], b_eq[:, :])
            G.dma_start(tb_t2[:], b_t2[:, :])
            G.dma_start(td_eq[:], d_eq[:, :])
            G.dma_start(ta_v[:, 0:3 * na], a_v[:, 0:3 * na])
            G.dma_start(ta_v[:, 3 * na:6 * na], a_v[:, 3 * na:6 * na])
            G.dma_start(ta_eq[:], a_eq[:, :])
            G.dma_start(ta_t2[:], a_t2[:, :])
            G.dma_start(td_v[:, 0:3 * nd], d_v[:, 0:3 * nd])
            G.dma_start(td_v[:, 3 * nd:6 * nd], d_v[:, 3 * nd:6 * nd])
            G.dma_start(td_v[:, 6 * nd:9 * nd], d_v[:, 6 * nd:9 * nd])

            bn2 = binp.tile([P, nb], F16, name="bn2")
            btmp = binp.tile([P, nb], F16, name="btmp")
            bd = binp.tile([P, nb], F16, name="bd")
            A.activation(bn2[:], tb_v[:, 0:nb], AF.Square)          # sqrt-set
            A.activation(btmp[:], tb_v[:, nb:2 * nb], AF.Square)
            V.tensor_add(bn2[:], bn2[:], btmp[:])
            A.activation(btmp[:], tb_v[:, 2 * nb:3 * nb], AF.Square)
            V.tensor_add(bn2[:], bn2[:], btmp[:])
            # d = sqrt(n2_scaled / SB^2) -> unscale inside activation
            A.activation(bd[:], bn2[:], AF.Sqrt, scale=1.0 / (SB * SB))
            V.tensor_sub(bd[:], bd[:], tb_eq[:])      # diff (in place)
            G.tensor_mul(bd[:], bd[:], bd[:])         # diff^2
            G.tensor_sub(bd[:], bd[:], tb_t2[:])      # - tol^2
            V.tensor_scalar(bd[:], bd[:], 0.0, None, ALU.max, ALU.add,
                            accum_out=acc[:, 0:1])    # relu + sum

        # ---- work planes (heavily reused; see per-stage comments) ----
        awrk = ctx.enter_context(tc.tile_pool(name="awrk", bufs=1))
        dwrk = ctx.enter_context(tc.tile_pool(name="dwrk", bufs=1))
        # angle planes [P, na] fp16
        aP0 = awrk.tile([P, na], F16, name="aP0")   # n0 / nn / aden / aa+diff
        aP1 = awrk.tile([P, na], F16, name="aP1")   # n1 / sqnn / aratio
        aP2 = awrk.tile([P, na], F16, name="aP2")   # tmp / m_ / ar2 / asq+t0+e
        aP3 = awrk.tile([P, na], F16, name="aP3")   # tmp / p_
        aP4 = awrk.tile([P, na], F16, name="aP4")   # d01 / q_
        aF0 = awrk.tile([P, na], F32, name="aF0")   # add_ ; dih X2->den->drt
        aF1 = awrk.tile([P, na], F32, name="aF1")   # arcp ; dih Y2->dinv->cdd
        # dih planes [P, nd] fp16
        dP = [dwrk.tile([P, nd], F16, name=f"dP{i}") for i in range(9)]

        # ================= ANGLES (front) =================
        # planes: b0 = ta_v[:, 0:3na], b1 = ta_v[:, 3na:6na]
        def apl(k):
            return ta_v[:, k * na:(k + 1) * na]
        # n0 -> aP0, n1 -> aP1 (ACT squares staged through aP2/aP3)
        A.activation(aP0[:], apl(0), AF.Square)                     # sqrt-set
        A.activation(aP2[:], apl(1), AF.Square)
        V.tensor_add(aP0[:], aP0[:], aP2[:])
        A.activation(aP2[:], apl(2), AF.Square)
        V.tensor_add(aP0[:], aP0[:], aP2[:])
        A.activation(aP1[:], apl(3), AF.Square)
        A.activation(aP3[:], apl(4), AF.Square)
        V.tensor_add(aP1[:], aP1[:], aP3[:])
        A.activation(aP3[:], apl(5), AF.Square)
        V.tensor_add(aP1[:], aP1[:], aP3[:])
        # d01 -> aP4 (scratch aP2)
        V.tensor_mul(aP2[:], apl(0), apl(3))
        V.tensor_mul(aP4[:], apl(1), apl(4))
        V.tensor_add(aP4[:], aP4[:], aP2[:])
        V.tensor_mul(aP2[:], apl(2), apl(5))
        V.tensor_add(aP4[:], aP4[:], aP2[:])
        # nn -> aP0 (in place), sqnn -> aP1
        V.tensor_mul(aP0[:], aP0[:], aP1[:])
        A.activation(aP1[:], aP0[:], AF.Sqrt)                       # sqrt-set
        # m_ = relu(sqnn - d01) -> aP2 ; p_ = sqrt(m_) -> aP3
        G.tensor_sub(aP2[:], aP1[:], aP4[:])
        V.tensor_scalar(aP2[:], aP2[:], 0.0, None, ALU.max)
        A.activation(aP3[:], aP2[:], AF.Sqrt)                       # sqrt-set
        # aden = relu(sqnn + d01) -> aP0 ; q_ = sqrt(aden) -> aP4
        V.tensor_add(aP0[:], aP1[:], aP4[:])
        V.tensor_scalar(aP0[:], aP0[:], 0.0, None, ALU.max)
        A.activation(aP4[:], aP0[:], AF.Sqrt)                       # sqrt-set
        # r2 = sqrt(2*sqnn) -> aP2
        A.activation(aP2[:], aP1[:], AF.Sqrt, scale=2.0)            # sqrt-set
        # add_ = max(r2 + q_, 1e-6) -> aF0 ; arcp = 1/add_ -> aF1
        V.tensor_add(aF0[:, 0:na], aP2[:], aP4[:])
        V.tensor_scalar(aF0[:, 0:na], aF0[:, 0:na], 1e-6, None, ALU.max)
        V.reciprocal_approx_fast(aF1[:, 0:na], aF0[:, 0:na])
        # aratio = p_ * arcp -> aP1
        V.tensor_mul(aP1[:], aP3[:], aF1[:, 0:na])

        # ================= DIHEDRALS (DVE/Pool mainline) =================
        # planes: b0 = td_v[:, 0:3nd], u = [3nd:6nd], b2 = [6nd:9nd]
        def dpl(k):
            return td_v[:, k * nd:(k + 1) * nd]
        b0x, b0y, b0z = dpl(0), dpl(1), dpl(2)
        ux, uy, uz = dpl(3), dpl(4), dpl(5)
        b2x, b2y, b2z = dpl(6), dpl(7), dpl(8)

        dL2, dm0, dm1 = dP[0], dP[1], dP[2]
        db0b2, db0u, db2u = dP[3], dP[4], dP[5]
        dY, gm0, dL = dP[6], dP[7], dP[8]

        # b0.u on Pool (scratch gm0)
        G.tensor_mul(gm0[:], b0x[:], ux[:])
        G.tensor_mul(db0u[:], b0y[:], uy[:])
        G.tensor_add(db0u[:], db0u[:], gm0[:])
        G.tensor_mul(gm0[:], b0z[:], uz[:])
        G.tensor_add(db0u[:], db0u[:], gm0[:])
        # L2 via ACT squares (staged through dm0)
        A.activation(dL2[:], ux[:], AF.Square)                      # sqrt-set
        A.activation(dm0[:], uy[:], AF.Square)
        V.tensor_add(dL2[:], dL2[:], dm0[:])
        A.activation(dm0[:], uz[:], AF.Square)
        V.tensor_add(dL2[:], dL2[:], dm0[:])
        # b0.b2 on DVE
        V.tensor_mul(dm0[:], b0x[:], b2x[:])
        V.tensor_mul(db0b2[:], b0y[:], b2y[:])
        V.tensor_add(db0b2[:], db0b2[:], dm0[:])
        V.tensor_mul(dm0[:], b0z[:], b2z[:])
        V.tensor_add(db0b2[:], db0b2[:], dm0[:])
        # b2.u on DVE
        V.tensor_mul(dm0[:], b2x[:], ux[:])
        V.tensor_mul(db2u[:], b2y[:], uy[:])
        V.tensor_add(db2u[:], db2u[:], dm0[:])
        V.tensor_mul(dm0[:], b2z[:], uz[:])
        V.tensor_add(db2u[:], db2u[:], dm0[:])
        # Y = (u x b0) . b2 -> dY (scratch dm0, dm1)
        V.tensor_mul(dm0[:], uy[:], b0z[:])
        V.tensor_mul(dm1[:], uz[:], b0y[:])
        V.tensor_sub(dm0[:], dm0[:], dm1[:])
        V.tensor_mul(dY[:], dm0[:], b2x[:])
        V.tensor_mul(dm0[:], uz[:], b0x[:])
        V.tensor_mul(dm1[:], ux[:], b0z[:])
        V.tensor_sub(dm0[:], dm0[:], dm1[:])
        V.tensor_mul(dm0[:], dm0[:], b2y[:])
        V.tensor_add(dY[:], dY[:], dm0[:])
        V.tensor_mul(dm0[:], ux[:], b0y[:])
        V.tensor_mul(dm1[:], uy[:], b0x[:])
        V.tensor_sub(dm0[:], dm0[:], dm1[:])
        V.tensor_mul(dm0[:], dm0[:], b2z[:])
        V.tensor_add(dY[:], dY[:], dm0[:])
        # X = L2*b0b2 - (b0.u)(b2.u) -> db0b2 ; t2 -> db0u
        V.tensor_mul(db0b2[:], dL2[:], db0b2[:])
        G.tensor_mul(db0u[:], db0u[:], db2u[:])
        V.tensor_sub(db0b2[:], db0b2[:], db0u[:])
        dX = db0b2
        # den = X^2 + L2*Y^2 (fp32; reuses angle aF0/aF1, free after arcp)
        fA = aF0[:, 0:nd]
        fB = aF1[:, 0:nd]
        A.activation(fA, dX[:], AF.Square)                          # sqrt-set
        A.activation(fB, dY[:], AF.Square)                          # sqrt-set
        V.tensor_mul(fB, dL2[:], fB)                # L2*Y^2
        V.tensor_add(fA, fA, fB)                    # den
        V.tensor_scalar(fA, fA, 1e-12, None, ALU.max)
        V.reciprocal_approx_fast(fB, fA)            # 1/den
        drt = fA
        A.activation(drt, fB, AF.Sqrt)              # rt = 1/sqrt(den)  sqrt-set
        A.activation(dL[:], dL2[:], AF.Sqrt)                        # sqrt-set

        # ---- trig-set phase ----
        dseq, daeq, dceq = db2u, dm1, dm0
        A.activation(dseq[:], td_eq[:], AF.Sin)                     # trig-set
        A.activation(daeq[:], td_eq[:], AF.Abs)
        A.activation(dceq[:], daeq[:], AF.Sin, scale=-1.0, bias=halfpi[:])
        # angle: a = atan(ratio); diff = 4a - eq; e = relu(diff^2 - tol^2)
        aa = aP0
        A.activation(aa[:], aP1[:], AF.Arctan)                      # trig-set
        V.scalar_tensor_tensor(aa[:], aa[:], 4.0, ta_eq[:],
                               ALU.mult, ALU.subtract)
        asq = aP2
        A.activation(asq[:], aa[:], AF.Square)                      # trig-set
        V.tensor_sub(asq[:], asq[:], ta_t2[:])
        V.tensor_scalar(asq[:], asq[:], 0.0, None, ALU.max, ALU.add,
                        accum_out=acc[:, 1:2])

        # ---- dihedral tail: cdd = (X*ceq + L*Y*seq) * rt ----
        V.tensor_mul(dY[:], dL[:], dY[:])           # LY
        V.tensor_mul(dX[:], dX[:], dceq[:])         # nx
        V.tensor_mul(dY[:], dY[:], dseq[:])         # ny
        V.tensor_add(dX[:], dX[:], dY[:])           # num
        dcdd = fB
        V.scalar_tensor_tensor(dcdd, dX[:], 1.0, drt,
                               ALU.mult, ALU.mult, accum_out=acc[:, 2:3])

        G.dma_start(partials[:], acc[:])
    nc.compile()
    return nc


def _run_spmd(nc, in_maps):
    import os
    if os.environ.get("EK_SIM") == "1":
        from concourse.bass_interp import CoreSim
        results = []
        for m in in_maps:
            sim = CoreSim(nc)
            for k, v in m.items():
                sim.tensor(k)[:] = v
            sim.simulate()
            results.append({"partials": np.array(sim.tensor("partials"))})
        return results
    from concourse.bass_utils import run_bass_kernel_spmd
    res = run_bass_kernel_spmd(nc, in_maps, list(range(len(in_maps))))
    return res.results


_BUILD_CACHE = {}


def _get_kernel(nb, na, nd):
    key = (nb, na, nd)
    if key not in _BUILD_CACHE:
        _BUILD_CACHE[key] = build_kernel(nb, na, nd)
    return _BUILD_CACHE[key]


def _shard_pad(arr, n_pad_per_core, fill=0.0):
    """[N,...] -> list of 8 per-core arrays padded to n_pad_per_core."""
    n = arr.shape[0]
    per = n // N_CORES
    out = []
    for c in range(N_CORES):
        a = arr[c * per:(c + 1) * per]
        npad = n_pad_per_core - per
        if npad:
            pad = np.full((npad,) + a.shape[1:], fill, dtype=a.dtype)
            a = np.concatenate([a, pad])
        out.append(a)
    return out


def _planes16(vecs, cols, ncomp):
    """[n_pad, ncomp] fp32 -> [P, ncomp*cols] fp16 planar."""
    v = vecs.reshape(P, cols, ncomp).transpose(0, 2, 1)  # [P, ncomp, cols]
    return np.ascontiguousarray(v.reshape(P, ncomp * cols).astype(np.float16))


def kernel(pos, bond_idcs, bond_eq_val, bond_tolerance,
           angle_idcs, angle_eq_val, angle_tolerance,
           dih_idcs, dih_eq_val):
    pos = np.asarray(pos, dtype=np.float32)
    bond_idcs = np.asarray(bond_idcs)
    angle_idcs = np.asarray(angle_idcs)
    dih_idcs = np.asarray(dih_idcs)

    nb = _cols(N_BONDS // N_CORES)
    na = _cols(N_ANGLES // N_CORES)
    nd = _cols(N_DIH // N_CORES)
    nbp, nap, ndp = nb * P, na * P, nd * P

    # ---- bonds: D = p0 - p1 (scaled) ----
    bD = (pos[bond_idcs[:, 0]] - pos[bond_idcs[:, 1]]) * SB
    b_eq = np.asarray(bond_eq_val, np.float32)
    b_t2 = np.asarray(bond_tolerance, np.float32) ** 2
    bDs = _shard_pad(bD, nbp)
    beqs = _shard_pad(b_eq, nbp)
    bt2s = _shard_pad(b_t2, nbp, fill=PAD_TOL2)

    # ---- angles: B0 = p0 - p1, B1 = p2 - p1 (scaled) ----
    aP1 = pos[angle_idcs[:, 1]]
    aB0 = (pos[angle_idcs[:, 0]] - aP1) * SA
    aB1 = (pos[angle_idcs[:, 2]] - aP1) * SA
    del aP1
    aV = np.concatenate([aB0, aB1], axis=1)  # [N,6]
    del aB0, aB1
    a_eq = np.asarray(angle_eq_val, np.float32)
    a_t2 = np.asarray(angle_tolerance, np.float32) ** 2
    aVs = _shard_pad(aV, nap)
    del aV
    aeqs = _shard_pad(a_eq, nap)
    at2s = _shard_pad(a_t2, nap, fill=PAD_TOL2)

    # ---- dihedrals: B0 = p0 - p1, U = p2 - p1, B2 = p3 - p2 (scaled) ----
    dP1 = pos[dih_idcs[:, 1]]
    dP2 = pos[dih_idcs[:, 2]]
    dB0 = (pos[dih_idcs[:, 0]] - dP1) * SD
    dU = (dP2 - dP1) * SD
    dB2 = (pos[dih_idcs[:, 3]] - dP2) * SD
    del dP1, dP2
    dV = np.concatenate([dB0, dU, dB2], axis=1)  # [N,9]
    del dB0, dU, dB2
    d_eq = np.asarray(dih_eq_val, np.float32)
    dVs = _shard_pad(dV, ndp)
    del dV
    deqs = _shard_pad(d_eq, ndp)

    nc = _get_kernel(nb, na, nd)

    in_maps = []
    for c in range(N_CORES):
        in_maps.append({
            "b_v": _planes16(bDs[c], nb, 3),
            "b_eq": beqs[c].reshape(P, nb).astype(np.float16),
            "b_t2": bt2s[c].reshape(P, nb).astype(np.float16),
            "a_v": _planes16(aVs[c], na, 6),
            "a_eq": aeqs[c].reshape(P, na).astype(np.float16),
            "a_t2": at2s[c].reshape(P, na).astype(np.float16),
            "d_v": _planes16(dVs[c], nd, 9),
            "d_eq": deqs[c].reshape(P, nd).astype(np.float16),
        })

    results = _run_spmd(nc, in_maps)

    bond_sum = 0.0
    angle_sum = 0.0
    cos_sum = 0.0
    for c in range(N_CORES):
        p = results[c]["partials"].astype(np.float64)
        bond_sum += p[:, 0].sum()
        angle_sum += p[:, 1].sum()
        cos_sum += p[:, 2].sum()

    # padding terms contribute exactly 0 to all three sums
    bond_energy = 1000.0 * bond_sum / N_BONDS
    angle_energy = 150.0 * angle_sum / N_ANGLES
    dih_energy = (2.0 * N_DIH - 2.0 * cos_sum) / N_DIH
    total = bond_energy + angle_energy + dih_energy
    return (np.float32(total), np.float32(bond_energy),
            np.float32(angle_energy), np.float32(dih_energy))


# revision 23
# speedup vs baseline: 2.5790x; 1.1330x over previous
"""Trainium2 Bass kernel for nn_MinimizeEnergy (bond/angle/dihedral energies).

Strategy: data-parallel over the term axis (8 cores, equal shards). Host
marshals the gather: per term it emits edge-difference vectors (p_i - p_j)
as scaled fp16 planes (planar SoA layout, one [P, cols] plane per vector
component), plus fp16 eq / tol^2 planes. The device kernel does all the
math: norms, half-angle arctan for bond angles, dihedral cos via the
X/Y trig-free formulation, energy terms, and per-partition accumulation.

Numerics: fp16 throughout the elementwise pipeline (DVE 2x mode), fp32
reductions. Vectors are pre-scaled (bonds/angles 1/16, dihedrals 1/32) so
all intermediates stay in fp16 range; the scales cancel in the angle/dih
ratios and are undone inside the bond sqrt's free scale slot.

ACT table sets are grouped into three phases (sqrt -> reciprocal -> trig)
to pay only three ACT_TABLE_LOADs.
"""
import sys
for _p in ('/opt/trn_rl_repo',):
    if _p not in sys.path:
        sys.path.insert(0, _p)

import numpy as np
from contextlib import ExitStack

import concourse.bass as bass
import concourse.tile as tile
from concourse import bacc, mybir

F32 = mybir.dt.float32
F16 = mybir.dt.float16
AF = mybir.ActivationFunctionType
ALU = mybir.AluOpType
AX = mybir.AxisListType
PI = float(np.pi)
P = 128
N_CORES = 8

N_ATOMS = 2_000_000
N_BONDS = 2_000_000
N_ANGLES = 4_000_000
N_DIH = 2_000_000

SB = 1.0 / 8.0    # bond vector prescale
SA = 1.0 / 8.0    # angle vector prescale
SD = 1.0 / 8.0    # dihedral vector prescale

PAD_TOL2 = 1.0e3  # tol^2 for padding terms -> relu(...) == 0


def _cols(n_per_core):
    """Columns per partition, padded so every plane is 4B-aligned (cols
    multiple of 4)."""
    c = -(-n_per_core // P)
    return -(-c // 4) * 4


def build_kernel(nb, na, nd, dbg=False):
    """nb/na/nd: per-core column counts (terms per partition). nb == nd
    is assumed by the buffer-sharing plan below."""
    nc = bacc.Bacc("TRN2", target_bir_lowering=False, debug=False,
                   num_devices=N_CORES)
    if dbg:
        g_ratio = nc.dram_tensor("g_ratio", [P, na], F16, kind="ExternalOutput").ap()
        g_diff = nc.dram_tensor("g_diff", [P, na], F16, kind="ExternalOutput").ap()
        g_sqnn = nc.dram_tensor("g_sqnn", [P, na], F16, kind="ExternalOutput").ap()
        g_m = nc.dram_tensor("g_m", [P, na], F16, kind="ExternalOutput").ap()
        g_p = nc.dram_tensor("g_p", [P, na], F16, kind="ExternalOutput").ap()
        g_aden = nc.dram_tensor("g_aden", [P, na], F16, kind="ExternalOutput").ap()
        g_q = nc.dram_tensor("g_q", [P, na], F16, kind="ExternalOutput").ap()
        g_r2 = nc.dram_tensor("g_r2", [P, na], F16, kind="ExternalOutput").ap()
        g_add = nc.dram_tensor("g_add", [P, na], F16, kind="ExternalOutput").ap()
        g_rcp = nc.dram_tensor("g_rcp", [P, na], F16, kind="ExternalOutput").ap()
    b_v = nc.dram_tensor("b_v", [P, 3 * nb], F16, kind="ExternalInput").ap()
    b_eq = nc.dram_tensor("b_eq", [P, nb], F16, kind="ExternalInput").ap()
    b_t2 = nc.dram_tensor("b_t2", [P, nb], F16, kind="ExternalInput").ap()
    a_v = nc.dram_tensor("a_v", [P, 6 * na], F16, kind="ExternalInput").ap()
    a_eq = nc.dram_tensor("a_eq", [P, na], F16, kind="ExternalInput").ap()
    a_t2 = nc.dram_tensor("a_t2", [P, na], F16, kind="ExternalInput").ap()
    d_v = nc.dram_tensor("d_v", [P, 9 * nd], F16, kind="ExternalInput").ap()
    d_eq = nc.dram_tensor("d_eq", [P, nd], F16, kind="ExternalInput").ap()
    partials = nc.dram_tensor("partials", [P, 4], F32, kind="ExternalOutput").ap()

    V = nc.vector      # DVE
    A = nc.scalar      # ACT
    G = nc.gpsimd      # Pool

    with tile.TileContext(nc) as tc, ExitStack() as ctx:
        pers = ctx.enter_context(tc.tile_pool(name="pers", bufs=1))
        ainp = ctx.enter_context(tc.tile_pool(name="ainp", bufs=1))
        dinp = ctx.enter_context(tc.tile_pool(name="dinp", bufs=1))

        acc = pers.tile([P, 4], F32)
        V.memset(acc[:], 0.0)
        halfpi = pers.tile([P, 1], F32)
        V.memset(halfpi[:], PI / 2)

        # ---- input tiles + DMA (issued up front; Pool engine triggers) ----
        ta_v = ainp.tile([P, 6 * na], F16)
        ta_eq = ainp.tile([P, na], F16)
        ta_t2 = ainp.tile([P, na], F16)
        td_v = dinp.tile([P, 9 * nd], F16)
        td_eq = dinp.tile([P, nd], F16)

        # ================= BONDS (own pool scope, freed after) =============
        with tc.tile_pool(name="binp", bufs=1) as binp:
            tb_v = binp.tile([P, 3 * nb], F16)
            tb_eq = binp.tile([P, nb], F16)
            tb_t2 = binp.tile([P, nb], F16)
            G.dma_start(tb_v[:], b_v[:, :])
            G.dma_start(tb_eq[:], b_eq[:, :])
            G.dma_start(tb_t2[:], b_t2[:, :])
            G.dma_start(td_eq[:], d_eq[:, :])
            G.dma_start(ta_v[:, 0:3 * na], a_v[:, 0:3 * na])
            G.dma_start(ta_v[:, 3 * na:6 * na], a_v[:, 3 * na:6 * na])
            G.dma_start(ta_eq[:], a_eq[:, :])
            G.dma_start(ta_t2[:], a_t2[:, :])
            G.dma_start(td_v[:, 0:3 * nd], d_v[:, 0:3 * nd])
            G.dma_start(td_v[:, 3 * nd:6 * nd], d_v[:, 3 * nd:6 * nd])
            G.dma_start(td_v[:, 6 * nd:9 * nd], d_v[:, 6 * nd:9 * nd])

            bn2 = binp.tile([P, nb], F16, name="bn2")
            btmp = binp.tile([P, nb], F16, name="btmp")
            bd = binp.tile([P, nb], F16, name="bd")
            A.activation(bn2[:], tb_v[:, 0:nb], AF.Square)          # sqrt-set
            A.activation(btmp[:], tb_v[:, nb:2 * nb], AF.Square)
            V.tensor_add(bn2[:], bn2[:], btmp[:])
            A.activation(btmp[:], tb_v[:, 2 * nb:3 * nb], AF.Square)
            V.tensor_add(bn2[:], bn2[:], btmp[:])
            # d = sqrt(n2_scaled / SB^2) -> unscale inside activation
            A.activation(bd[:], bn2[:], AF.Sqrt, scale=1.0 / (SB * SB))
            V.tensor_sub(bd[:], bd[:], tb_eq[:])      # diff (in place)
            G.tensor_mul(bd[:], bd[:], bd[:])         # diff^2
            G.tensor_sub(bd[:], bd[:], tb_t2[:])      # - tol^2
            V.tensor_scalar(bd[:], bd[:], 0.0, None, ALU.max, ALU.add,
                            accum_out=acc[:, 0:1])    # relu + sum

        # ---- work planes (heavily reused; see per-stage comments) ----
        awrk = ctx.enter_context(tc.tile_pool(name="awrk", bufs=1))
        dwrk = ctx.enter_context(tc.tile_pool(name="dwrk", bufs=1))
        # angle planes [P, na] fp16
        aP0 = awrk.tile([P, na], F16, name="aP0")   # n0 / nn / aden / aa+diff
        aP1 = awrk.tile([P, na], F16, name="aP1")   # n1 / sqnn / aratio
        aP2 = awrk.tile([P, na], F16, name="aP2")   # tmp / m_ / ar2 / asq+t0+e
        aP3 = awrk.tile([P, na], F16, name="aP3")   # tmp / p_
        aP4 = awrk.tile([P, na], F16, name="aP4")   # d01 / q_
        aF0 = awrk.tile([P, na], F32, name="aF0")   # add_ ; dih X2->den->drt
        aF1 = awrk.tile([P, na], F32, name="aF1")   # arcp ; dih Y2->dinv->cdd
        # dih planes [P, nd] fp16
        dP = [dwrk.tile([P, nd], F16, name=f"dP{i}") for i in range(9)]

        # ================= ANGLES (front) =================
        # planes: b0 = ta_v[:, 0:3na], b1 = ta_v[:, 3na:6na]
        def apl(k):
            return ta_v[:, k * na:(k + 1) * na]
        # n0 -> aP0, n1 -> aP1 (ACT squares staged through aP2/aP3)
        A.activation(aP0[:], apl(0), AF.Square)                     # sqrt-set
        A.activation(aP2[:], apl(1), AF.Square)
        V.tensor_add(aP0[:], aP0[:], aP2[:])
        A.activation(aP2[:], apl(2), AF.Square)
        V.tensor_add(aP0[:], aP0[:], aP2[:])
        A.activation(aP1[:], apl(3), AF.Square)
        A.activation(aP3[:], apl(4), AF.Square)
        V.tensor_add(aP1[:], aP1[:], aP3[:])
        A.activation(aP3[:], apl(5), AF.Square)
        V.tensor_add(aP1[:], aP1[:], aP3[:])
        # d01 -> aP4 (scratch aP2)
        V.tensor_mul(aP2[:], apl(0), apl(3))
        V.tensor_mul(aP4[:], apl(1), apl(4))
        V.tensor_add(aP4[:], aP4[:], aP2[:])
        V.tensor_mul(aP2[:], apl(2), apl(5))
        V.tensor_add(aP4[:], aP4[:], aP2[:])
        # nn -> aP0 (in place), sqnn -> aP1
        V.tensor_mul(aP0[:], aP0[:], aP1[:])
        A.activation(aP1[:], aP0[:], AF.Sqrt)                       # sqrt-set
        if dbg:
            G.dma_start(g_sqnn[:], aP1[:])
        # m_ = relu(sqnn - d01) -> aP2 ; p_ = sqrt(m_) -> aP3
        G.tensor_sub(aP2[:], aP1[:], aP4[:])
        V.tensor_scalar(aP2[:], aP2[:], 0.0, None, ALU.max)
        if dbg:
            G.dma_start(g_m[:], aP2[:])
        A.activation(aP3[:], aP2[:], AF.Sqrt)                       # sqrt-set
        if dbg:
            G.dma_start(g_p[:], aP3[:])
        # aden = relu(sqnn + d01) -> aP0 ; q_ = sqrt(aden) -> aP4
        V.tensor_add(aP0[:], aP1[:], aP4[:])
        V.tensor_scalar(aP0[:], aP0[:], 0.0, None, ALU.max)
        if dbg:
            G.dma_start(g_aden[:], aP0[:])
        A.activation(aP4[:], aP0[:], AF.Sqrt)
        if dbg:
            G.dma_start(g_q[:], aP4[:])                       # sqrt-set
        # r2 = sqrt(2*sqnn) -> aP2
        A.activation(aP2[:], aP1[:], AF.Sqrt, scale=2.0)            # sqrt-set
        if dbg:
            G.dma_start(g_r2[:], aP2[:])
        # add_ = max(r2 + q_, 1e-6); fp16 add, ACT-copy to fp32 for recip
        V.tensor_add(aP0[:], aP2[:], aP4[:])
        if dbg:
            G.dma_start(g_add[:], aP0[:])
        A.activation(aF0[:, 0:na], aP0[:], AF.Copy)
        V.tensor_scalar(aF0[:, 0:na], aF0[:, 0:na], 1e-4, None, ALU.max)
        V.reciprocal_approx_fast(aF1[:, 0:na], aF0[:, 0:na])
        A.activation(aP4[:], aF1[:, 0:na], AF.Copy)   # arcp -> fp16 (q_ dead)
        if dbg:
            G.dma_start(g_rcp[:], aP4[:])
        # aratio = min(p_ * arcp, 1) -> aP1  (atan domain guarantee)
        V.tensor_mul(aP1[:], aP3[:], aP4[:])
        V.tensor_scalar(aP1[:], aP1[:], 1.0, None, ALU.min)

        # ================= DIHEDRALS (DVE/Pool mainline) =================
        # planes: b0 = td_v[:, 0:3nd], u = [3nd:6nd], b2 = [6nd:9nd]
        def dpl(k):
            return td_v[:, k * nd:(k + 1) * nd]
        b0x, b0y, b0z = dpl(0), dpl(1), dpl(2)
        ux, uy, uz = dpl(3), dpl(4), dpl(5)
        b2x, b2y, b2z = dpl(6), dpl(7), dpl(8)

        dL2, dm0, dm1 = dP[0], dP[1], dP[2]
        db0b2, db0u, db2u = dP[3], dP[4], dP[5]
        dY, gm0, dL = dP[6], dP[7], dP[8]

        # b0.u on Pool (scratch gm0)
        G.tensor_mul(gm0[:], b0x[:], ux[:])
        G.tensor_mul(db0u[:], b0y[:], uy[:])
        G.tensor_add(db0u[:], db0u[:], gm0[:])
        G.tensor_mul(gm0[:], b0z[:], uz[:])
        G.tensor_add(db0u[:], db0u[:], gm0[:])
        # L2 via ACT squares (staged through dm0)
        A.activation(dL2[:], ux[:], AF.Square)                      # sqrt-set
        A.activation(dm0[:], uy[:], AF.Square)
        V.tensor_add(dL2[:], dL2[:], dm0[:])
        A.activation(dm0[:], uz[:], AF.Square)
        V.tensor_add(dL2[:], dL2[:], dm0[:])
        # b0.b2 on DVE
        V.tensor_mul(dm0[:], b0x[:], b2x[:])
        V.tensor_mul(db0b2[:], b0y[:], b2y[:])
        V.tensor_add(db0b2[:], db0b2[:], dm0[:])
        V.tensor_mul(dm0[:], b0z[:], b2z[:])
        V.tensor_add(db0b2[:], db0b2[:], dm0[:])
        # b2.u on DVE
        V.tensor_mul(dm0[:], b2x[:], ux[:])
        V.tensor_mul(db2u[:], b2y[:], uy[:])
        V.tensor_add(db2u[:], db2u[:], dm0[:])
        V.tensor_mul(dm0[:], b2z[:], uz[:])
        V.tensor_add(db2u[:], db2u[:], dm0[:])
        # Y = (u x b0) . b2 -> dY (scratch dm0, dm1)
        V.tensor_mul(dm0[:], uy[:], b0z[:])
        V.tensor_mul(dm1[:], uz[:], b0y[:])
        V.tensor_sub(dm0[:], dm0[:], dm1[:])
        V.tensor_mul(dY[:], dm0[:], b2x[:])
        V.tensor_mul(dm0[:], uz[:], b0x[:])
        V.tensor_mul(dm1[:], ux[:], b0z[:])
        V.tensor_sub(dm0[:], dm0[:], dm1[:])
        V.tensor_mul(dm0[:], dm0[:], b2y[:])
        V.tensor_add(dY[:], dY[:], dm0[:])
        V.tensor_mul(dm0[:], ux[:], b0y[:])
        V.tensor_mul(dm1[:], uy[:], b0x[:])
        V.tensor_sub(dm0[:], dm0[:], dm1[:])
        V.tensor_mul(dm0[:], dm0[:], b2z[:])
        V.tensor_add(dY[:], dY[:], dm0[:])
        # X = L2*b0b2 - (b0.u)(b2.u) -> db0b2 ; t2 -> db0u
        V.tensor_mul(db0b2[:], dL2[:], db0b2[:])
        G.tensor_mul(db0u[:], db0u[:], db2u[:])
        V.tensor_sub(db0b2[:], db0b2[:], db0u[:])
        dX = db0b2
        # den = X^2 + (L*Y)^2 (fp32; reuses angle aF0/aF1, free after arcp)
        A.activation(dL[:], dL2[:], AF.Sqrt)                        # sqrt-set
        V.tensor_mul(dY[:], dL[:], dY[:])           # LY (fp16, homogeneous)
        fA = aF0[:, 0:nd]
        fB = aF1[:, 0:nd]
        A.activation(fA, dX[:], AF.Square)                          # sqrt-set
        A.activation(fB, dY[:], AF.Square)          # (L*Y)^2          sqrt-set
        V.tensor_add(fA, fA, fB)                    # den
        V.tensor_scalar(fA, fA, 1e-9, None, ALU.max)
        V.reciprocal_approx_fast(fB, fA)            # 1/den
        A.activation(fA, fB, AF.Sqrt)               # 1/sqrt(den)     sqrt-set
        drt16 = dL2                                 # dL2 dead after dL
        A.activation(drt16[:], fA, AF.Copy)         # rt -> fp16

        # ---- trig-set phase ----
        dseq, daeq, dceq = db2u, dm1, dm0
        A.activation(dseq[:], td_eq[:], AF.Sin)                     # trig-set
        A.activation(daeq[:], td_eq[:], AF.Abs)
        A.activation(dceq[:], daeq[:], AF.Sin, scale=-1.0, bias=halfpi[:])
        # angle: a = atan(ratio); diff = 4a - eq; e = relu(diff^2 - tol^2)
        aa = aP0
        A.activation(aa[:], aP1[:], AF.Arctan)                      # trig-set
        V.scalar_tensor_tensor(aa[:], aa[:], 4.0, ta_eq[:],
                               ALU.mult, ALU.subtract)
        if dbg:
            G.dma_start(g_ratio[:], aP1[:])
            G.dma_start(g_diff[:], aa[:])
        asq = aP2
        A.activation(asq[:], aa[:], AF.Square)                      # trig-set
        V.tensor_sub(asq[:], asq[:], ta_t2[:])
        V.tensor_scalar(asq[:], asq[:], 0.0, None, ALU.max, ALU.add,
                        accum_out=acc[:, 1:2])

        # ---- dihedral tail: cdd = (X*ceq + LY*seq) * rt  (all fp16) ----
        V.tensor_mul(dX[:], dX[:], dceq[:])         # nx
        V.tensor_mul(dY[:], dY[:], dseq[:])         # ny
        V.tensor_add(dX[:], dX[:], dY[:])           # num
        dcdd = dm0                                  # ceq consumed by nx
        V.scalar_tensor_tensor(dcdd[:], dX[:], 1.0, drt16[:],
                               ALU.mult, ALU.mult, accum_out=acc[:, 2:3])

        G.dma_start(partials[:], acc[:])
    nc.compile()
    return nc


def _run_spmd(nc, in_maps):
    import os
    if os.environ.get("EK_SIM") == "1":
        from concourse.bass_interp import CoreSim
        results = []
        for m in in_maps:
            sim = CoreSim(nc)
            for k, v in m.items():
                sim.tensor(k)[:] = v
            sim.simulate()
            results.append({"partials": np.array(sim.tensor("partials"))})
        return results
    from concourse.bass_utils import run_bass_kernel_spmd
    res = run_bass_kernel_spmd(nc, in_maps, list(range(len(in_maps))))
    return res.results


_BUILD_CACHE = {}


def _get_kernel(nb, na, nd):
    key = (nb, na, nd)
    if key not in _BUILD_CACHE:
        _BUILD_CACHE[key] = build_kernel(nb, na, nd)
    return _BUILD_CACHE[key]


def _shard_pad(arr, n_pad_per_core, fill=0.0):
    """[N,...] -> list of 8 per-core arrays padded to n_pad_per_core."""
    n = arr.shape[0]
    per = n // N_CORES
    out = []
    for c in range(N_CORES):
        a = arr[c * per:(c + 1) * per]
        npad = n_pad_per_core - per
        if npad:
            pad = np.full((npad,) + a.shape[1:], fill, dtype=a.dtype)
            a = np.concatenate([a, pad])
        out.append(a)
    return out


def _planes16(vecs, cols, ncomp):
    """[n_pad, ncomp] fp32 -> [P, ncomp*cols] fp16 planar."""
    v = vecs.reshape(P, cols, ncomp).transpose(0, 2, 1)  # [P, ncomp, cols]
    return np.ascontiguousarray(v.reshape(P, ncomp * cols).astype(np.float16))


def kernel(pos, bond_idcs, bond_eq_val, bond_tolerance,
           angle_idcs, angle_eq_val, angle_tolerance,
           dih_idcs, dih_eq_val):
    pos = np.asarray(pos, dtype=np.float32)
    bond_idcs = np.asarray(bond_idcs)
    angle_idcs = np.asarray(angle_idcs)
    dih_idcs = np.asarray(dih_idcs)

    nb = _cols(N_BONDS // N_CORES)
    na = _cols(N_ANGLES // N_CORES)
    nd = _cols(N_DIH // N_CORES)
    nbp, nap, ndp = nb * P, na * P, nd * P

    # ---- bonds: D = p0 - p1 (scaled) ----
    bD = (pos[bond_idcs[:, 0]] - pos[bond_idcs[:, 1]]) * SB
    b_eq = np.asarray(bond_eq_val, np.float32)
    b_t2 = np.asarray(bond_tolerance, np.float32) ** 2
    bDs = _shard_pad(bD, nbp)
    beqs = _shard_pad(b_eq, nbp)
    bt2s = _shard_pad(b_t2, nbp, fill=PAD_TOL2)

    # ---- angles: B0 = p0 - p1, B1 = p2 - p1 (scaled) ----
    aP1 = pos[angle_idcs[:, 1]]
    aB0 = (pos[angle_idcs[:, 0]] - aP1) * SA
    aB1 = (pos[angle_idcs[:, 2]] - aP1) * SA
    del aP1
    aV = np.concatenate([aB0, aB1], axis=1)  # [N,6]
    del aB0, aB1
    a_eq = np.asarray(angle_eq_val, np.float32)
    a_t2 = np.asarray(angle_tolerance, np.float32) ** 2
    aVs = _shard_pad(aV, nap)
    del aV
    aeqs = _shard_pad(a_eq, nap)
    at2s = _shard_pad(a_t2, nap, fill=PAD_TOL2)

    # ---- dihedrals: B0 = p0 - p1, U = p2 - p1, B2 = p3 - p2 (scaled) ----
    dP1 = pos[dih_idcs[:, 1]]
    dP2 = pos[dih_idcs[:, 2]]
    dB0 = (pos[dih_idcs[:, 0]] - dP1) * SD
    dU = (dP2 - dP1) * SD
    dB2 = (pos[dih_idcs[:, 3]] - dP2) * SD
    del dP1, dP2
    dV = np.concatenate([dB0, dU, dB2], axis=1)  # [N,9]
    del dB0, dU, dB2
    d_eq = np.asarray(dih_eq_val, np.float32)
    dVs = _shard_pad(dV, ndp)
    del dV
    deqs = _shard_pad(d_eq, ndp)

    nc = _get_kernel(nb, na, nd)

    in_maps = []
    for c in range(N_CORES):
        in_maps.append({
            "b_v": _planes16(bDs[c], nb, 3),
            "b_eq": beqs[c].reshape(P, nb).astype(np.float16),
            "b_t2": bt2s[c].reshape(P, nb).astype(np.float16),
            "a_v": _planes16(aVs[c], na, 6),
            "a_eq": aeqs[c].reshape(P, na).astype(np.float16),
            "a_t2": at2s[c].reshape(P, na).astype(np.float16),
            "d_v": _planes16(dVs[c], nd, 9),
            "d_eq": deqs[c].reshape(P, nd).astype(np.float16),
        })

    results = _run_spmd(nc, in_maps)

    bond_sum = 0.0
    angle_sum = 0.0
    cos_sum = 0.0
    for c in range(N_CORES):
        p = results[c]["partials"].astype(np.float64)
        bond_sum += p[:, 0].sum()
        angle_sum += p[:, 1].sum()
        cos_sum += p[:, 2].sum()

    # padding terms contribute exactly 0 to all three sums
    bond_energy = 1000.0 * bond_sum / N_BONDS
    angle_energy = 150.0 * angle_sum / N_ANGLES
    dih_energy = (2.0 * N_DIH - 2.0 * cos_sum) / N_DIH
    total = bond_energy + angle_energy + dih_energy
    return (np.float32(total), np.float32(bond_energy),
            np.float32(angle_energy), np.float32(dih_energy))


# revision 24
# speedup vs baseline: 2.9627x; 1.1488x over previous
"""Trainium2 Bass kernel for nn_MinimizeEnergy (bond/angle/dihedral energies).

Strategy: data-parallel over the term axis (8 cores, equal shards). Host
marshals the gather: per term it emits edge-difference vectors (p_i - p_j)
as scaled fp16 planes (planar SoA layout, one [P, cols] plane per vector
component), plus fp16 eq / tol^2 planes. The device kernel does all the
math: norms, half-angle arctan for bond angles, dihedral cos via the
X/Y trig-free formulation, energy terms, and per-partition accumulation.

Numerics: fp16 throughout the elementwise pipeline (DVE 2x mode), fp32
reductions. Vectors are pre-scaled (bonds/angles 1/16, dihedrals 1/32) so
all intermediates stay in fp16 range; the scales cancel in the angle/dih
ratios and are undone inside the bond sqrt's free scale slot.

ACT table sets are grouped into three phases (sqrt -> reciprocal -> trig)
to pay only three ACT_TABLE_LOADs.
"""
import sys
for _p in ('/opt/trn_rl_repo',):
    if _p not in sys.path:
        sys.path.insert(0, _p)

import numpy as np
from contextlib import ExitStack

import concourse.bass as bass
import concourse.tile as tile
from concourse import bacc, mybir

F32 = mybir.dt.float32
F16 = mybir.dt.float16
AF = mybir.ActivationFunctionType
ALU = mybir.AluOpType
AX = mybir.AxisListType
PI = float(np.pi)
P = 128
N_CORES = 8

N_ATOMS = 2_000_000
N_BONDS = 2_000_000
N_ANGLES = 4_000_000
N_DIH = 2_000_000

SB = 1.0 / 8.0    # bond vector prescale
SA = 1.0 / 8.0    # angle vector prescale
SD = 1.0 / 8.0    # dihedral vector prescale

PAD_TOL2 = 1.0e3  # tol^2 for padding terms -> relu(...) == 0


def _cols(n_per_core):
    """Columns per partition, padded so every plane is 4B-aligned (cols
    multiple of 4)."""
    c = -(-n_per_core // P)
    return -(-c // 4) * 4


def build_kernel(nb, na, nd, dbg=False):
    """nb/na/nd: per-core column counts (terms per partition). nb == nd
    is assumed by the buffer-sharing plan below."""
    nc = bacc.Bacc("TRN2", target_bir_lowering=False, debug=False,
                   num_devices=N_CORES)
    if dbg:
        g_ratio = nc.dram_tensor("g_ratio", [P, na], F16, kind="ExternalOutput").ap()
        g_diff = nc.dram_tensor("g_diff", [P, na], F16, kind="ExternalOutput").ap()
        g_sqnn = nc.dram_tensor("g_sqnn", [P, na], F16, kind="ExternalOutput").ap()
        g_m = nc.dram_tensor("g_m", [P, na], F16, kind="ExternalOutput").ap()
        g_p = nc.dram_tensor("g_p", [P, na], F16, kind="ExternalOutput").ap()
        g_aden = nc.dram_tensor("g_aden", [P, na], F16, kind="ExternalOutput").ap()
        g_q = nc.dram_tensor("g_q", [P, na], F16, kind="ExternalOutput").ap()
        g_r2 = nc.dram_tensor("g_r2", [P, na], F16, kind="ExternalOutput").ap()
        g_add = nc.dram_tensor("g_add", [P, na], F16, kind="ExternalOutput").ap()
        g_rcp = nc.dram_tensor("g_rcp", [P, na], F16, kind="ExternalOutput").ap()
    b_v = nc.dram_tensor("b_v", [P, 3 * nb], F16, kind="ExternalInput").ap()
    b_eq = nc.dram_tensor("b_eq", [P, nb], F16, kind="ExternalInput").ap()
    b_t2 = nc.dram_tensor("b_t2", [P, nb], F16, kind="ExternalInput").ap()
    a_v = nc.dram_tensor("a_v", [P, 6 * na], F16, kind="ExternalInput").ap()
    a_eq = nc.dram_tensor("a_eq", [P, na], F16, kind="ExternalInput").ap()
    a_t2 = nc.dram_tensor("a_t2", [P, na], F16, kind="ExternalInput").ap()
    d_v = nc.dram_tensor("d_v", [P, 9 * nd], F16, kind="ExternalInput").ap()
    d_eq = nc.dram_tensor("d_eq", [P, nd], F16, kind="ExternalInput").ap()
    partials = nc.dram_tensor("partials", [P, 4], F32, kind="ExternalOutput").ap()

    V = nc.vector      # DVE
    A = nc.scalar      # ACT
    G = nc.gpsimd      # Pool

    with tile.TileContext(nc) as tc, ExitStack() as ctx:
        pers = ctx.enter_context(tc.tile_pool(name="pers", bufs=1))
        ainp = ctx.enter_context(tc.tile_pool(name="ainp", bufs=1))
        dinp = ctx.enter_context(tc.tile_pool(name="dinp", bufs=1))

        acc = pers.tile([P, 4], F32)
        V.memset(acc[:], 0.0)
        halfpi = pers.tile([P, 1], F32)
        V.memset(halfpi[:], PI / 2)

        # ---- input tiles + DMA (issued up front; Pool engine triggers) ----
        ta_v = ainp.tile([P, 6 * na], F16)
        ta_eq = ainp.tile([P, na], F16)
        ta_t2 = ainp.tile([P, na], F16)
        td_v = dinp.tile([P, 9 * nd], F16)
        td_eq = dinp.tile([P, nd], F16)

        # ================= BONDS (own pool scope, freed after) =============
        with tc.tile_pool(name="binp", bufs=1) as binp:
            tb_v = binp.tile([P, 3 * nb], F16)
            tb_eq = binp.tile([P, nb], F16)
            tb_t2 = binp.tile([P, nb], F16)
            G.dma_start(tb_v[:], b_v[:, :])
            G.dma_start(tb_eq[:], b_eq[:, :])
            G.dma_start(tb_t2[:], b_t2[:, :])
            G.dma_start(td_eq[:], d_eq[:, :])
            G.dma_start(ta_v[:, 0:3 * na], a_v[:, 0:3 * na])
            G.dma_start(ta_v[:, 3 * na:6 * na], a_v[:, 3 * na:6 * na])
            G.dma_start(ta_eq[:], a_eq[:, :])
            G.dma_start(ta_t2[:], a_t2[:, :])
            G.dma_start(td_v[:, 0:3 * nd], d_v[:, 0:3 * nd])
            G.dma_start(td_v[:, 3 * nd:6 * nd], d_v[:, 3 * nd:6 * nd])
            G.dma_start(td_v[:, 6 * nd:9 * nd], d_v[:, 6 * nd:9 * nd])

            bn2 = binp.tile([P, nb], F16, name="bn2")
            btmp = binp.tile([P, nb], F16, name="btmp")
            bd = binp.tile([P, nb], F16, name="bd")
            bx = tb_v[:, 0:nb]
            by = tb_v[:, nb:2 * nb]
            bz = tb_v[:, 2 * nb:3 * nb]
            V.tensor_mul(bn2[:], bx, bx)
            V.tensor_mul(btmp[:], by, by)
            V.tensor_add(bn2[:], bn2[:], btmp[:])
            V.tensor_mul(btmp[:], bz, bz)
            V.tensor_add(bn2[:], bn2[:], btmp[:])
            # d = sqrt(n2_scaled / SB^2) -> unscale inside activation
            A.activation(bd[:], bn2[:], AF.Sqrt, scale=1.0 / (SB * SB))
            V.tensor_sub(bd[:], bd[:], tb_eq[:])      # diff (in place)
            V.tensor_mul(bd[:], bd[:], bd[:])         # diff^2
            V.tensor_sub(bd[:], bd[:], tb_t2[:])      # - tol^2
            V.tensor_scalar(bd[:], bd[:], 0.0, None, ALU.max, ALU.add,
                            accum_out=acc[:, 0:1])    # relu + sum

        # ---- work planes (heavily reused; see per-stage comments) ----
        awrk = ctx.enter_context(tc.tile_pool(name="awrk", bufs=1))
        dwrk = ctx.enter_context(tc.tile_pool(name="dwrk", bufs=1))
        # angle planes [P, na] fp16
        aP0 = awrk.tile([P, na], F16, name="aP0")   # n0 / nn / aden / aa+diff
        aP1 = awrk.tile([P, na], F16, name="aP1")   # n1 / sqnn / aratio
        aP2 = awrk.tile([P, na], F16, name="aP2")   # tmp / m_ / ar2 / asq+t0+e
        aP3 = awrk.tile([P, na], F16, name="aP3")   # tmp / p_
        aP4 = awrk.tile([P, na], F16, name="aP4")   # d01 / q_
        aF0 = awrk.tile([P, na], F32, name="aF0")   # add_ ; dih X2->den->drt
        aF1 = awrk.tile([P, na], F32, name="aF1")   # arcp ; dih Y2->dinv->cdd
        # dih planes [P, nd] fp16
        dP = [dwrk.tile([P, nd], F16, name=f"dP{i}") for i in range(9)]

        # ================= ANGLES (front) =================
        # planes: b0 = ta_v[:, 0:3na], b1 = ta_v[:, 3na:6na]
        def apl(k):
            return ta_v[:, k * na:(k + 1) * na]
        # n0 -> aP0, n1 -> aP1 (ACT squares staged through aP2/aP3)
        A.activation(aP0[:], apl(0), AF.Square)                     # sqrt-set
        A.activation(aP2[:], apl(1), AF.Square)
        V.tensor_add(aP0[:], aP0[:], aP2[:])
        A.activation(aP2[:], apl(2), AF.Square)
        V.tensor_add(aP0[:], aP0[:], aP2[:])
        A.activation(aP1[:], apl(3), AF.Square)
        A.activation(aP3[:], apl(4), AF.Square)
        V.tensor_add(aP1[:], aP1[:], aP3[:])
        A.activation(aP3[:], apl(5), AF.Square)
        V.tensor_add(aP1[:], aP1[:], aP3[:])
        # d01 -> aP4 (scratch aP2)
        V.tensor_mul(aP2[:], apl(0), apl(3))
        V.tensor_mul(aP4[:], apl(1), apl(4))
        V.tensor_add(aP4[:], aP4[:], aP2[:])
        V.tensor_mul(aP2[:], apl(2), apl(5))
        V.tensor_add(aP4[:], aP4[:], aP2[:])
        # nn -> aP0 (in place), sqnn -> aP1
        V.tensor_mul(aP0[:], aP0[:], aP1[:])
        A.activation(aP1[:], aP0[:], AF.Sqrt)                       # sqrt-set
        if dbg:
            G.dma_start(g_sqnn[:], aP1[:])
        # m_ = relu(sqnn - d01) -> aP2 ; p_ = sqrt(m_) -> aP3
        V.tensor_sub(aP2[:], aP1[:], aP4[:])
        V.tensor_scalar(aP2[:], aP2[:], 0.0, None, ALU.max)
        if dbg:
            G.dma_start(g_m[:], aP2[:])
        A.activation(aP3[:], aP2[:], AF.Sqrt)                       # sqrt-set
        if dbg:
            G.dma_start(g_p[:], aP3[:])
        # aden = relu(sqnn + d01) -> aP0 ; q_ = sqrt(aden) -> aP4
        V.tensor_add(aP0[:], aP1[:], aP4[:])
        V.tensor_scalar(aP0[:], aP0[:], 0.0, None, ALU.max)
        if dbg:
            G.dma_start(g_aden[:], aP0[:])
        A.activation(aP4[:], aP0[:], AF.Sqrt)
        if dbg:
            G.dma_start(g_q[:], aP4[:])                       # sqrt-set
        # r2 = sqrt(2*sqnn) -> aP2
        A.activation(aP2[:], aP1[:], AF.Sqrt, scale=2.0)            # sqrt-set
        if dbg:
            G.dma_start(g_r2[:], aP2[:])
        # add_ = (r2 + 1e-4) + q_ -> fp32 (fused eps keeps recip finite)
        V.scalar_tensor_tensor(aF0[:, 0:na], aP2[:], 1e-4, aP4[:],
                               ALU.add, ALU.add)
        if dbg:
            G.dma_start(g_add[:], aP2[:])
        V.reciprocal_approx_fast(aF1[:, 0:na], aF0[:, 0:na])
        if dbg:
            G.dma_start(g_rcp[:], aP4[:])
        # aratio = min(p_ * arcp, 1) -> aP1  (atan domain guarantee)
        V.tensor_mul(aP1[:], aP3[:], aF1[:, 0:na])
        V.tensor_scalar(aP1[:], aP1[:], 1.0, None, ALU.min)

        # ================= DIHEDRALS (DVE/Pool mainline) =================
        # planes: b0 = td_v[:, 0:3nd], u = [3nd:6nd], b2 = [6nd:9nd]
        def dpl(k):
            return td_v[:, k * nd:(k + 1) * nd]
        b0x, b0y, b0z = dpl(0), dpl(1), dpl(2)
        ux, uy, uz = dpl(3), dpl(4), dpl(5)
        b2x, b2y, b2z = dpl(6), dpl(7), dpl(8)

        dL2, dm0, dm1 = dP[0], dP[1], dP[2]
        db0b2, db0u, db2u = dP[3], dP[4], dP[5]
        dY, gm0, dL = dP[6], dP[7], dP[8]

        # b0.u on Pool (scratch gm0)
        G.tensor_mul(gm0[:], b0x[:], ux[:])
        G.tensor_mul(db0u[:], b0y[:], uy[:])
        G.tensor_add(db0u[:], db0u[:], gm0[:])
        G.tensor_mul(gm0[:], b0z[:], uz[:])
        G.tensor_add(db0u[:], db0u[:], gm0[:])
        # L2 via ACT squares (staged through dm0)
        A.activation(dL2[:], ux[:], AF.Square)                      # sqrt-set
        A.activation(dm0[:], uy[:], AF.Square)
        V.tensor_add(dL2[:], dL2[:], dm0[:])
        A.activation(dm0[:], uz[:], AF.Square)
        V.tensor_add(dL2[:], dL2[:], dm0[:])
        # b0.b2 on DVE
        V.tensor_mul(dm0[:], b0x[:], b2x[:])
        V.tensor_mul(db0b2[:], b0y[:], b2y[:])
        V.tensor_add(db0b2[:], db0b2[:], dm0[:])
        V.tensor_mul(dm0[:], b0z[:], b2z[:])
        V.tensor_add(db0b2[:], db0b2[:], dm0[:])
        # b2.u on DVE
        V.tensor_mul(dm0[:], b2x[:], ux[:])
        V.tensor_mul(db2u[:], b2y[:], uy[:])
        V.tensor_add(db2u[:], db2u[:], dm0[:])
        V.tensor_mul(dm0[:], b2z[:], uz[:])
        V.tensor_add(db2u[:], db2u[:], dm0[:])
        # Y = (u x b0) . b2 -> dY (scratch dm0, dm1)
        V.tensor_mul(dm0[:], uy[:], b0z[:])
        V.tensor_mul(dm1[:], uz[:], b0y[:])
        V.tensor_sub(dm0[:], dm0[:], dm1[:])
        V.tensor_mul(dY[:], dm0[:], b2x[:])
        V.tensor_mul(dm0[:], uz[:], b0x[:])
        V.tensor_mul(dm1[:], ux[:], b0z[:])
        V.tensor_sub(dm0[:], dm0[:], dm1[:])
        V.tensor_mul(dm0[:], dm0[:], b2y[:])
        V.tensor_add(dY[:], dY[:], dm0[:])
        V.tensor_mul(dm0[:], ux[:], b0y[:])
        V.tensor_mul(dm1[:], uy[:], b0x[:])
        V.tensor_sub(dm0[:], dm0[:], dm1[:])
        V.tensor_mul(dm0[:], dm0[:], b2z[:])
        V.tensor_add(dY[:], dY[:], dm0[:])
        # X = L2*b0b2 - (b0.u)(b2.u) -> db0b2 ; t2 -> db0u
        V.tensor_mul(db0b2[:], dL2[:], db0b2[:])
        G.tensor_mul(db0u[:], db0u[:], db2u[:])
        V.tensor_sub(db0b2[:], db0b2[:], db0u[:])
        dX = db0b2
        # den = X^2 + (L*Y)^2 (fp32; reuses angle aF0/aF1, free after arcp)
        A.activation(dL[:], dL2[:], AF.Sqrt)                        # sqrt-set
        V.tensor_mul(dY[:], dL[:], dY[:])           # LY (fp16, homogeneous)
        fA = aF0[:, 0:nd]
        fB = aF1[:, 0:nd]
        A.activation(fA, dX[:], AF.Square)                          # sqrt-set
        A.activation(fB, dY[:], AF.Square)          # (L*Y)^2          sqrt-set
        V.scalar_tensor_tensor(fA, fA, 1e-9, fB, ALU.add, ALU.add)  # den+eps
        V.reciprocal_approx_fast(fB, fA)            # 1/den
        drt16 = dL2                                 # dL2 dead after dL
        A.activation(drt16[:], fB, AF.Sqrt)         # 1/sqrt(den) -> fp16

        # ---- trig-set phase ----
        dseq, daeq, dceq = db2u, dm1, dm0
        A.activation(dseq[:], td_eq[:], AF.Sin)                     # trig-set
        A.activation(daeq[:], td_eq[:], AF.Abs)
        A.activation(dceq[:], daeq[:], AF.Sin, scale=-1.0, bias=halfpi[:])
        # angle: a = atan(ratio); diff = 4a - eq; e = relu(diff^2 - tol^2)
        aa = aP0
        A.activation(aa[:], aP1[:], AF.Arctan)                      # trig-set
        V.scalar_tensor_tensor(aa[:], aa[:], 4.0, ta_eq[:],
                               ALU.mult, ALU.subtract)
        if dbg:
            G.dma_start(g_ratio[:], aP1[:])
            G.dma_start(g_diff[:], aa[:])
        asq = aP2
        A.activation(asq[:], aa[:], AF.Square)                      # trig-set
        V.tensor_sub(asq[:], asq[:], ta_t2[:])
        V.tensor_scalar(asq[:], asq[:], 0.0, None, ALU.max, ALU.add,
                        accum_out=acc[:, 1:2])

        # ---- dihedral tail: cdd = (X*ceq + LY*seq) * rt  (all fp16) ----
        V.tensor_mul(dX[:], dX[:], dceq[:])         # nx
        V.tensor_mul(dY[:], dY[:], dseq[:])         # ny
        V.tensor_add(dX[:], dX[:], dY[:])           # num
        dcdd = dm0                                  # ceq consumed by nx
        V.scalar_tensor_tensor(dcdd[:], dX[:], 1.0, drt16[:],
                               ALU.mult, ALU.mult, accum_out=acc[:, 2:3])

        G.dma_start(partials[:], acc[:])
    nc.compile()
    return nc


def _run_spmd(nc, in_maps):
    import os
    if os.environ.get("EK_SIM") == "1":
        from concourse.bass_interp import CoreSim
        results = []
        for m in in_maps:
            sim = CoreSim(nc)
            for k, v in m.items():
                sim.tensor(k)[:] = v
            sim.simulate()
            results.append({"partials": np.array(sim.tensor("partials"))})
        return results
    from concourse.bass_utils import run_bass_kernel_spmd
    res = run_bass_kernel_spmd(nc, in_maps, list(range(len(in_maps))))
    return res.results


_BUILD_CACHE = {}


def _get_kernel(nb, na, nd):
    key = (nb, na, nd)
    if key not in _BUILD_CACHE:
        _BUILD_CACHE[key] = build_kernel(nb, na, nd)
    return _BUILD_CACHE[key]


def _shard_pad(arr, n_pad_per_core, fill=0.0):
    """[N,...] -> list of 8 per-core arrays padded to n_pad_per_core."""
    n = arr.shape[0]
    per = n // N_CORES
    out = []
    for c in range(N_CORES):
        a = arr[c * per:(c + 1) * per]
        npad = n_pad_per_core - per
        if npad:
            pad = np.full((npad,) + a.shape[1:], fill, dtype=a.dtype)
            a = np.concatenate([a, pad])
        out.append(a)
    return out


def _planes16(vecs, cols, ncomp):
    """[n_pad, ncomp] fp32 -> [P, ncomp*cols] fp16 planar."""
    v = vecs.reshape(P, cols, ncomp).transpose(0, 2, 1)  # [P, ncomp, cols]
    return np.ascontiguousarray(v.reshape(P, ncomp * cols).astype(np.float16))


def kernel(pos, bond_idcs, bond_eq_val, bond_tolerance,
           angle_idcs, angle_eq_val, angle_tolerance,
           dih_idcs, dih_eq_val):
    pos = np.asarray(pos, dtype=np.float32)
    bond_idcs = np.asarray(bond_idcs)
    angle_idcs = np.asarray(angle_idcs)
    dih_idcs = np.asarray(dih_idcs)

    nb = _cols(N_BONDS // N_CORES)
    na = _cols(N_ANGLES // N_CORES)
    nd = _cols(N_DIH // N_CORES)
    nbp, nap, ndp = nb * P, na * P, nd * P

    # ---- bonds: D = p0 - p1 (scaled) ----
    bD = (pos[bond_idcs[:, 0]] - pos[bond_idcs[:, 1]]) * SB
    b_eq = np.asarray(bond_eq_val, np.float32)
    b_t2 = np.asarray(bond_tolerance, np.float32) ** 2
    bDs = _shard_pad(bD, nbp)
    beqs = _shard_pad(b_eq, nbp)
    bt2s = _shard_pad(b_t2, nbp, fill=PAD_TOL2)

    # ---- angles: B0 = p0 - p1, B1 = p2 - p1 (scaled) ----
    aP1 = pos[angle_idcs[:, 1]]
    aB0 = (pos[angle_idcs[:, 0]] - aP1) * SA
    aB1 = (pos[angle_idcs[:, 2]] - aP1) * SA
    del aP1
    aV = np.concatenate([aB0, aB1], axis=1)  # [N,6]
    del aB0, aB1
    a_eq = np.asarray(angle_eq_val, np.float32)
    a_t2 = np.asarray(angle_tolerance, np.float32) ** 2
    aVs = _shard_pad(aV, nap)
    del aV
    aeqs = _shard_pad(a_eq, nap)
    at2s = _shard_pad(a_t2, nap, fill=PAD_TOL2)

    # ---- dihedrals: B0 = p0 - p1, U = p2 - p1, B2 = p3 - p2 (scaled) ----
    dP1 = pos[dih_idcs[:, 1]]
    dP2 = pos[dih_idcs[:, 2]]
    dB0 = (pos[dih_idcs[:, 0]] - dP1) * SD
    dU = (dP2 - dP1) * SD
    dB2 = (pos[dih_idcs[:, 3]] - dP2) * SD
    del dP1, dP2
    dV = np.concatenate([dB0, dU, dB2], axis=1)  # [N,9]
    del dB0, dU, dB2
    d_eq = np.asarray(dih_eq_val, np.float32)
    dVs = _shard_pad(dV, ndp)
    del dV
    deqs = _shard_pad(d_eq, ndp)

    nc = _get_kernel(nb, na, nd)

    in_maps = []
    for c in range(N_CORES):
        in_maps.append({
            "b_v": _planes16(bDs[c], nb, 3),
            "b_eq": beqs[c].reshape(P, nb).astype(np.float16),
            "b_t2": bt2s[c].reshape(P, nb).astype(np.float16),
            "a_v": _planes16(aVs[c], na, 6),
            "a_eq": aeqs[c].reshape(P, na).astype(np.float16),
            "a_t2": at2s[c].reshape(P, na).astype(np.float16),
            "d_v": _planes16(dVs[c], nd, 9),
            "d_eq": deqs[c].reshape(P, nd).astype(np.float16),
        })

    results = _run_spmd(nc, in_maps)

    bond_sum = 0.0
    angle_sum = 0.0
    cos_sum = 0.0
    for c in range(N_CORES):
        p = results[c]["partials"].astype(np.float64)
        bond_sum += p[:, 0].sum()
        angle_sum += p[:, 1].sum()
        cos_sum += p[:, 2].sum()

    # padding terms contribute exactly 0 to all three sums
    bond_energy = 1000.0 * bond_sum / N_BONDS
    angle_energy = 150.0 * angle_sum / N_ANGLES
    dih_energy = (2.0 * N_DIH - 2.0 * cos_sum) / N_DIH
    total = bond_energy + angle_energy + dih_energy
    return (np.float32(total), np.float32(bond_energy),
            np.float32(angle_energy), np.float32(dih_energy))


# revision 25
# speedup vs baseline: 3.0185x; 1.0188x over previous
"""Trainium2 Bass kernel for nn_MinimizeEnergy (bond/angle/dihedral energies).

Strategy: data-parallel over the term axis (8 cores, equal shards). Host
marshals the gather: per term it emits edge-difference vectors (p_i - p_j)
as scaled fp16 planes (planar SoA layout, one [P, cols] plane per vector
component), plus fp16 eq / tol^2 planes. The device kernel does all the
math: norms, half-angle arctan for bond angles, dihedral cos via the
X/Y trig-free formulation, energy terms, and per-partition accumulation.

Numerics: fp16 throughout the elementwise pipeline (DVE 2x mode), fp32
reductions. Vectors are pre-scaled (bonds/angles 1/16, dihedrals 1/32) so
all intermediates stay in fp16 range; the scales cancel in the angle/dih
ratios and are undone inside the bond sqrt's free scale slot.

ACT table sets are grouped into three phases (sqrt -> reciprocal -> trig)
to pay only three ACT_TABLE_LOADs.
"""
import sys
for _p in ('/opt/trn_rl_repo',):
    if _p not in sys.path:
        sys.path.insert(0, _p)

import numpy as np
from contextlib import ExitStack

import concourse.bass as bass
import concourse.tile as tile
from concourse import bacc, mybir

F32 = mybir.dt.float32
F16 = mybir.dt.float16
AF = mybir.ActivationFunctionType
ALU = mybir.AluOpType
AX = mybir.AxisListType
PI = float(np.pi)
P = 128
N_CORES = 8

N_ATOMS = 2_000_000
N_BONDS = 2_000_000
N_ANGLES = 4_000_000
N_DIH = 2_000_000

SB = 1.0 / 8.0    # bond vector prescale
SA = 1.0 / 8.0    # angle vector prescale
SD = 1.0 / 8.0    # dihedral vector prescale

PAD_TOL2 = 1.0e3  # tol^2 for padding terms -> relu(...) == 0


def _cols(n_per_core):
    """Columns per partition, padded so every plane is 4B-aligned (cols
    multiple of 4)."""
    c = -(-n_per_core // P)
    return -(-c // 4) * 4


def build_kernel(nb, na, nd, dbg=False):
    """nb/na/nd: per-core column counts (terms per partition).
    Angle/dihedral pipelines run in 2 column-chunks so the per-chunk
    dependency chains overlap across engines."""
    nc = bacc.Bacc("TRN2", target_bir_lowering=False, debug=False,
                   num_devices=N_CORES)
    b_v = nc.dram_tensor("b_v", [P, 3 * nb], F16, kind="ExternalInput").ap()
    b_eq = nc.dram_tensor("b_eq", [P, nb], F16, kind="ExternalInput").ap()
    b_t2 = nc.dram_tensor("b_t2", [P, nb], F16, kind="ExternalInput").ap()
    a_v = nc.dram_tensor("a_v", [P, 6 * na], F16, kind="ExternalInput").ap()
    a_eq = nc.dram_tensor("a_eq", [P, na], F16, kind="ExternalInput").ap()
    a_t2 = nc.dram_tensor("a_t2", [P, na], F16, kind="ExternalInput").ap()
    d_v = nc.dram_tensor("d_v", [P, 9 * nd], F16, kind="ExternalInput").ap()
    d_eq = nc.dram_tensor("d_eq", [P, nd], F16, kind="ExternalInput").ap()
    partials = nc.dram_tensor("partials", [P, 8], F32, kind="ExternalOutput").ap()

    V = nc.vector      # DVE
    A = nc.scalar      # ACT
    G = nc.gpsimd      # Pool

    NC_A = 2           # angle chunks
    NC_D = 2           # dih chunks
    ha = na // NC_A
    hd = nd // NC_D

    with tile.TileContext(nc) as tc, ExitStack() as ctx:
        pers = ctx.enter_context(tc.tile_pool(name="pers", bufs=1))
        ainp = ctx.enter_context(tc.tile_pool(name="ainp", bufs=1))
        dinp = ctx.enter_context(tc.tile_pool(name="dinp", bufs=1))

        acc = pers.tile([P, 8], F32)
        V.memset(acc[:], 0.0)
        halfpi = pers.tile([P, 1], F32)
        V.memset(halfpi[:], PI / 2)

        ta_v = ainp.tile([P, 6 * na], F16)
        ta_eq = ainp.tile([P, na], F16)
        ta_t2 = ainp.tile([P, na], F16)
        td_v = dinp.tile([P, 9 * nd], F16)
        td_eq = dinp.tile([P, nd], F16)

        # ================= BONDS (own pool scope, freed after) =============
        with tc.tile_pool(name="binp", bufs=1) as binp:
            tb_v = binp.tile([P, 3 * nb], F16)
            tb_eq = binp.tile([P, nb], F16)
            tb_t2 = binp.tile([P, nb], F16)
            G.dma_start(tb_v[:], b_v[:, :])
            G.dma_start(tb_eq[:], b_eq[:, :])
            G.dma_start(tb_t2[:], b_t2[:, :])
            G.dma_start(td_eq[:], d_eq[:, :])
            G.dma_start(ta_v[:, 0:3 * na], a_v[:, 0:3 * na])
            G.dma_start(ta_v[:, 3 * na:6 * na], a_v[:, 3 * na:6 * na])
            G.dma_start(ta_eq[:], a_eq[:, :])
            G.dma_start(ta_t2[:], a_t2[:, :])
            G.dma_start(td_v[:, 0:3 * nd], d_v[:, 0:3 * nd])
            G.dma_start(td_v[:, 3 * nd:6 * nd], d_v[:, 3 * nd:6 * nd])
            G.dma_start(td_v[:, 6 * nd:9 * nd], d_v[:, 6 * nd:9 * nd])

            bn2 = binp.tile([P, nb], F16, name="bn2")
            btmp = binp.tile([P, nb], F16, name="btmp")
            bd = binp.tile([P, nb], F16, name="bd")
            bx = tb_v[:, 0:nb]
            by = tb_v[:, nb:2 * nb]
            bz = tb_v[:, 2 * nb:3 * nb]
            V.tensor_mul(bn2[:], bx, bx)
            V.tensor_mul(btmp[:], by, by)
            V.tensor_add(bn2[:], bn2[:], btmp[:])
            V.tensor_mul(btmp[:], bz, bz)
            V.tensor_add(bn2[:], bn2[:], btmp[:])
            A.activation(bd[:], bn2[:], AF.Sqrt, scale=1.0 / (SB * SB))
            V.tensor_sub(bd[:], bd[:], tb_eq[:])      # diff
            V.tensor_mul(bd[:], bd[:], bd[:])         # diff^2
            V.tensor_sub(bd[:], bd[:], tb_t2[:])      # - tol^2
            V.tensor_scalar(bd[:], bd[:], 0.0, None, ALU.max, ALU.add,
                            accum_out=acc[:, 0:1])    # relu + sum

        awrk = ctx.enter_context(tc.tile_pool(name="awrk", bufs=1))
        dwrk = ctx.enter_context(tc.tile_pool(name="dwrk", bufs=1))
        aP0 = awrk.tile([P, na], F16, name="aP0")   # n0 / nn / aden / aa
        aP1 = awrk.tile([P, na], F16, name="aP1")   # n1 / sqnn / ratio
        aP2 = awrk.tile([P, na], F16, name="aP2")   # tmp / m_ / r2 / asq
        aP3 = awrk.tile([P, na], F16, name="aP3")   # tmp / p_
        aP4 = awrk.tile([P, na], F16, name="aP4")   # d01 / q_
        aF0 = awrk.tile([P, na], F32, name="aF0")   # add_ ; dih den
        aF1 = awrk.tile([P, na], F32, name="aF1")   # arcp ; dih 1/den
        dP = [dwrk.tile([P, nd], F16, name=f"dP{i}") for i in range(9)]

        # ---------------- angle stages (per column-chunk) ----------------
        def a_s(ci):
            return slice(ci * ha, (ci + 1) * ha)

        def apl(k, s):
            return ta_v[:, k * na + s.start:k * na + s.stop]

        def angle_front(ci):
            s = a_s(ci)
            p0, p1, p2, p3, p4 = (t[:, s] for t in (aP0, aP1, aP2, aP3, aP4))
            f0, f1 = aF0[:, s], aF1[:, s]
            A.activation(p0, apl(0, s), AF.Square)                  # sqrt-set
            A.activation(p2, apl(1, s), AF.Square)
            V.tensor_add(p0, p0, p2)
            A.activation(p2, apl(2, s), AF.Square)
            V.tensor_add(p0, p0, p2)
            A.activation(p1, apl(3, s), AF.Square)
            A.activation(p3, apl(4, s), AF.Square)
            V.tensor_add(p1, p1, p3)
            A.activation(p3, apl(5, s), AF.Square)
            V.tensor_add(p1, p1, p3)
            V.tensor_mul(p2, apl(0, s), apl(3, s))
            V.tensor_mul(p4, apl(1, s), apl(4, s))
            V.tensor_add(p4, p4, p2)
            V.tensor_mul(p2, apl(2, s), apl(5, s))
            V.tensor_add(p4, p4, p2)                 # d01
            V.tensor_mul(p0, p0, p1)                 # nn
            A.activation(p1, p0, AF.Sqrt)            # sqnn          sqrt-set
            V.tensor_sub(p2, p1, p4)                 # m_
            V.tensor_scalar(p2, p2, 0.0, None, ALU.max)
            A.activation(p3, p2, AF.Sqrt)            # p_            sqrt-set
            V.tensor_add(p0, p1, p4)                 # aden
            V.tensor_scalar(p0, p0, 0.0, None, ALU.max)
            A.activation(p4, p0, AF.Sqrt)            # q_            sqrt-set
            A.activation(p2, p1, AF.Sqrt, scale=2.0)  # r2           sqrt-set
            V.scalar_tensor_tensor(f0, p2, 1e-4, p4, ALU.add, ALU.add)
            V.reciprocal_approx_fast(f1, f0)
            V.tensor_mul(p1, p3, f1)                 # ratio (f16*f32->f16)
            V.tensor_scalar(p1, p1, 1.0, None, ALU.min)

        def angle_tail(ci):
            s = a_s(ci)
            p0, p1, p2 = aP0[:, s], aP1[:, s], aP2[:, s]
            A.activation(p0, p1, AF.Arctan)                         # trig-set
            V.scalar_tensor_tensor(p0, p0, 4.0, ta_eq[:, s],
                                   ALU.mult, ALU.subtract)
            A.activation(p2, p0, AF.Square)                         # trig-set
            V.tensor_sub(p2, p2, ta_t2[:, s])
            V.tensor_scalar(p2, p2, 0.0, None, ALU.max, ALU.add,
                            accum_out=acc[:, 1 + ci:2 + ci])

        # ---------------- dih stages (per column-chunk) ----------------
        def d_s(ci):
            return slice(ci * hd, (ci + 1) * hd)

        def dpl(k, s):
            return td_v[:, k * nd + s.start:k * nd + s.stop]

        def dih_main(ci):
            s = d_s(ci)
            b0x, b0y, b0z = dpl(0, s), dpl(1, s), dpl(2, s)
            ux, uy, uz = dpl(3, s), dpl(4, s), dpl(5, s)
            b2x, b2y, b2z = dpl(6, s), dpl(7, s), dpl(8, s)
            dL2, dm0, dm1 = dP[0][:, s], dP[1][:, s], dP[2][:, s]
            db0b2, db0u, db2u = dP[3][:, s], dP[4][:, s], dP[5][:, s]
            dY, gm0, dL = dP[6][:, s], dP[7][:, s], dP[8][:, s]
            fA, fB = aF0[:, s], aF1[:, s]
            # b0.u on Pool
            G.tensor_mul(gm0, b0x, ux)
            G.tensor_mul(db0u, b0y, uy)
            G.tensor_add(db0u, db0u, gm0)
            G.tensor_mul(gm0, b0z, uz)
            G.tensor_add(db0u, db0u, gm0)
            # L2 via ACT squares (staged through dm0)
            A.activation(dL2, ux, AF.Square)                        # sqrt-set
            A.activation(dm0, uy, AF.Square)
            V.tensor_add(dL2, dL2, dm0)
            A.activation(dm0, uz, AF.Square)
            V.tensor_add(dL2, dL2, dm0)
            # b0.b2 on DVE
            V.tensor_mul(dm0, b0x, b2x)
            V.tensor_mul(db0b2, b0y, b2y)
            V.tensor_add(db0b2, db0b2, dm0)
            V.tensor_mul(dm0, b0z, b2z)
            V.tensor_add(db0b2, db0b2, dm0)
            # b2.u on DVE
            V.tensor_mul(dm0, b2x, ux)
            V.tensor_mul(db2u, b2y, uy)
            V.tensor_add(db2u, db2u, dm0)
            V.tensor_mul(dm0, b2z, uz)
            V.tensor_add(db2u, db2u, dm0)
            # Y = (u x b0) . b2
            V.tensor_mul(dm0, uy, b0z)
            V.tensor_mul(dm1, uz, b0y)
            V.tensor_sub(dm0, dm0, dm1)
            V.tensor_mul(dY, dm0, b2x)
            V.tensor_mul(dm0, uz, b0x)
            V.tensor_mul(dm1, ux, b0z)
            V.tensor_sub(dm0, dm0, dm1)
            V.tensor_mul(dm0, dm0, b2y)
            V.tensor_add(dY, dY, dm0)
            V.tensor_mul(dm0, ux, b0y)
            V.tensor_mul(dm1, uy, b0x)
            V.tensor_sub(dm0, dm0, dm1)
            V.tensor_mul(dm0, dm0, b2z)
            V.tensor_add(dY, dY, dm0)
            # X = L2*b0b2 - (b0.u)(b2.u)
            V.tensor_mul(db0b2, dL2, db0b2)
            G.tensor_mul(db0u, db0u, db2u)
            V.tensor_sub(db0b2, db0b2, db0u)         # X
            # den = X^2 + (L*Y)^2, rt = 1/sqrt(den) -> fp16
            A.activation(dL, dL2, AF.Sqrt)                          # sqrt-set
            V.tensor_mul(dY, dL, dY)                 # LY
            A.activation(fA, db0b2, AF.Square)                      # sqrt-set
            A.activation(fB, dY, AF.Square)                         # sqrt-set
            V.scalar_tensor_tensor(fA, fA, 1e-9, fB, ALU.add, ALU.add)
            V.reciprocal_approx_fast(fB, fA)
            A.activation(dL2, fB, AF.Sqrt)           # rt16          sqrt-set

        def dih_trig(ci):
            s = d_s(ci)
            dm0, dm1, db2u = dP[1][:, s], dP[2][:, s], dP[5][:, s]
            A.activation(db2u, td_eq[:, s], AF.Sin)                 # trig-set
            A.activation(dm1, td_eq[:, s], AF.Abs)
            A.activation(dm0, dm1, AF.Sin, scale=-1.0, bias=halfpi[:])

        def dih_tail(ci):
            s = d_s(ci)
            dm0, db2u = dP[1][:, s], dP[5][:, s]
            dX, dY, rt16 = dP[3][:, s], dP[6][:, s], dP[0][:, s]
            V.tensor_mul(dX, dX, dm0)                # nx = X*ceq
            V.tensor_mul(dY, dY, db2u)               # ny = LY*seq
            V.tensor_add(dX, dX, dY)                 # num
            V.scalar_tensor_tensor(dX, dX, 1.0, rt16,
                                   ALU.mult, ALU.mult,
                                   accum_out=acc[:, 3 + ci:4 + ci])

        for ci in range(NC_A):
            angle_front(ci)
        for ci in range(NC_D):
            dih_main(ci)
        for ci in range(NC_D):
            dih_trig(ci)
        for ci in range(NC_A):
            angle_tail(ci)
        for ci in range(NC_D):
            dih_tail(ci)

        G.dma_start(partials[:], acc[:])
    nc.compile()
    return nc


def _run_spmd(nc, in_maps):
    import os
    if os.environ.get("EK_SIM") == "1":
        from concourse.bass_interp import CoreSim
        results = []
        for m in in_maps:
            sim = CoreSim(nc)
            for k, v in m.items():
                sim.tensor(k)[:] = v
            sim.simulate()
            results.append({"partials": np.array(sim.tensor("partials"))})
        return results
    from concourse.bass_utils import run_bass_kernel_spmd
    res = run_bass_kernel_spmd(nc, in_maps, list(range(len(in_maps))))
    return res.results


_BUILD_CACHE = {}


def _get_kernel(nb, na, nd):
    key = (nb, na, nd)
    if key not in _BUILD_CACHE:
        _BUILD_CACHE[key] = build_kernel(nb, na, nd)
    return _BUILD_CACHE[key]


def _shard_pad(arr, n_pad_per_core, fill=0.0):
    """[N,...] -> list of 8 per-core arrays padded to n_pad_per_core."""
    n = arr.shape[0]
    per = n // N_CORES
    out = []
    for c in range(N_CORES):
        a = arr[c * per:(c + 1) * per]
        npad = n_pad_per_core - per
        if npad:
            pad = np.full((npad,) + a.shape[1:], fill, dtype=a.dtype)
            a = np.concatenate([a, pad])
        out.append(a)
    return out


def _planes16(vecs, cols, ncomp):
    """[n_pad, ncomp] fp32 -> [P, ncomp*cols] fp16 planar."""
    v = vecs.reshape(P, cols, ncomp).transpose(0, 2, 1)  # [P, ncomp, cols]
    return np.ascontiguousarray(v.reshape(P, ncomp * cols).astype(np.float16))


def kernel(pos, bond_idcs, bond_eq_val, bond_tolerance,
           angle_idcs, angle_eq_val, angle_tolerance,
           dih_idcs, dih_eq_val):
    pos = np.asarray(pos, dtype=np.float32)
    bond_idcs = np.asarray(bond_idcs)
    angle_idcs = np.asarray(angle_idcs)
    dih_idcs = np.asarray(dih_idcs)

    nb = _cols(N_BONDS // N_CORES)
    na = _cols(N_ANGLES // N_CORES)
    nd = _cols(N_DIH // N_CORES)
    nbp, nap, ndp = nb * P, na * P, nd * P

    # ---- bonds: D = p0 - p1 (scaled) ----
    bD = (pos[bond_idcs[:, 0]] - pos[bond_idcs[:, 1]]) * SB
    b_eq = np.asarray(bond_eq_val, np.float32)
    b_t2 = np.asarray(bond_tolerance, np.float32) ** 2
    bDs = _shard_pad(bD, nbp)
    beqs = _shard_pad(b_eq, nbp)
    bt2s = _shard_pad(b_t2, nbp, fill=PAD_TOL2)

    # ---- angles: B0 = p0 - p1, B1 = p2 - p1 (scaled) ----
    aP1 = pos[angle_idcs[:, 1]]
    aB0 = (pos[angle_idcs[:, 0]] - aP1) * SA
    aB1 = (pos[angle_idcs[:, 2]] - aP1) * SA
    del aP1
    aV = np.concatenate([aB0, aB1], axis=1)  # [N,6]
    del aB0, aB1
    a_eq = np.asarray(angle_eq_val, np.float32)
    a_t2 = np.asarray(angle_tolerance, np.float32) ** 2
    aVs = _shard_pad(aV, nap)
    del aV
    aeqs = _shard_pad(a_eq, nap)
    at2s = _shard_pad(a_t2, nap, fill=PAD_TOL2)

    # ---- dihedrals: B0 = p0 - p1, U = p2 - p1, B2 = p3 - p2 (scaled) ----
    dP1 = pos[dih_idcs[:, 1]]
    dP2 = pos[dih_idcs[:, 2]]
    dB0 = (pos[dih_idcs[:, 0]] - dP1) * SD
    dU = (dP2 - dP1) * SD
    dB2 = (pos[dih_idcs[:, 3]] - dP2) * SD
    del dP1, dP2
    dV = np.concatenate([dB0, dU, dB2], axis=1)  # [N,9]
    del dB0, dU, dB2
    d_eq = np.asarray(dih_eq_val, np.float32)
    dVs = _shard_pad(dV, ndp)
    del dV
    deqs = _shard_pad(d_eq, ndp)

    nc = _get_kernel(nb, na, nd)

    in_maps = []
    for c in range(N_CORES):
        in_maps.append({
            "b_v": _planes16(bDs[c], nb, 3),
            "b_eq": beqs[c].reshape(P, nb).astype(np.float16),
            "b_t2": bt2s[c].reshape(P, nb).astype(np.float16),
            "a_v": _planes16(aVs[c], na, 6),
            "a_eq": aeqs[c].reshape(P, na).astype(np.float16),
            "a_t2": at2s[c].reshape(P, na).astype(np.float16),
            "d_v": _planes16(dVs[c], nd, 9),
            "d_eq": deqs[c].reshape(P, nd).astype(np.float16),
        })

    results = _run_spmd(nc, in_maps)

    bond_sum = 0.0
    angle_sum = 0.0
    cos_sum = 0.0
    for c in range(N_CORES):
        p = results[c]["partials"].astype(np.float64)
        bond_sum += p[:, 0].sum()
        angle_sum += p[:, 1].sum() + p[:, 2].sum()
        cos_sum += p[:, 3].sum() + p[:, 4].sum()

    # padding terms contribute exactly 0 to all three sums
    bond_energy = 1000.0 * bond_sum / N_BONDS
    angle_energy = 150.0 * angle_sum / N_ANGLES
    dih_energy = (2.0 * N_DIH - 2.0 * cos_sum) / N_DIH
    total = bond_energy + angle_energy + dih_energy
    return (np.float32(total), np.float32(bond_energy),
            np.float32(angle_energy), np.float32(dih_energy))
